# revision 1
# baseline (speedup 1.0000x reference)
"""Grouped-query attention, tensor-parallel over heads across 8 TRN2 NeuronCores.

Problem (hardcoded): x[1,1024,4096] @ Wq/Wk/Wv -> RoPE -> causal GQA
(32 q heads, 8 kv groups, head_dim 128) -> out proj Wo -> [1,1024,4096].

Sharding: core r owns q heads 4r..4r+3 and kv group r (Wq/Wk/Wv column
shards, Wo row shard). Each core computes a full [1024,4096] partial of
the output projection; the host sums the 8 partials (the "all-reduce").

Device kernel (per core, all matmuls bf16 with fp32 PSUM accumulation):
  qT[hd,s] = sum_c Wq_c^T x_c      (transposed layouts throughout; the
  kT[hd,s], v[t,hd]                 host ships x pre-transposed so no
  RoPE via permutation-matmul + DVE  on-device transposes are needed)
  ST[t,s] = khat_tile^T qhat       (causal: skip fully-masked tiles)
  P = exp(ST) * mask01             (1/sqrt(128) folded into Wq on host)
  den[1,s] = ones^T P;  ctxT[hd,s] = sum_t v^T P
  ctxhat = ctxT * (1/den broadcast) (DRAM-bounce partition broadcast)
  out[s,:] += ctxhat_h^T Wo_h       (accumulate 4 heads in PSUM)
"""

import numpy as np
import ml_dtypes

import concourse.bass as bass
import concourse.bacc as bacc
import concourse.mybir as mybir
import concourse.tile as tile
from concourse.bass_utils import run_bass_kernel_spmd

S = 1024          # sequence length
D = 4096          # model dim
H = 32            # query heads (global)
G = 8             # kv groups (global)
HD = 128          # head dim
N_CORES = 8
HPC = H // N_CORES   # 4 query heads per core
QW = HPC * HD        # 512 q-proj cols per core
NDC = D // 128       # 32 contraction chunks
BF = mybir.dt.bfloat16
F32 = mybir.dt.float32

_CACHE = {}


def _t_tiles(j):
    """Causal t-tile list for the 512-wide s-tile j, with mask index or None."""
    out = []
    for i in range(4 * j + 4):
        lo = i - 4 * j          # 128*i <= 512*j + ls needs mask when i-4j in 0..3
        out.append((i, lo if 0 <= lo <= 3 else None))
    return out


def _build():
    nc = bacc.Bacc("TRN2", target_bir_lowering=False, debug=False,
                   num_devices=N_CORES)

    xT = nc.dram_tensor("xT", [128, NDC, S], BF, kind="ExternalInput")
    wq = nc.dram_tensor("wq", [128, NDC, QW], BF, kind="ExternalInput")
    wk = nc.dram_tensor("wk", [128, NDC, HD], BF, kind="ExternalInput")
    wv = nc.dram_tensor("wv", [128, NDC, HD], BF, kind="ExternalInput")
    wo = nc.dram_tensor("wo", [128, HPC, D], BF, kind="ExternalInput")
    cosT = nc.dram_tensor("cosT", [HD, S], BF, kind="ExternalInput")
    sinT = nc.dram_tensor("sinT", [HD, S], BF, kind="ExternalInput")
    rmat = nc.dram_tensor("rmat", [HD, HD], BF, kind="ExternalInput")
    masks = nc.dram_tensor("masks", [128, 4, 512], BF, kind="ExternalInput")
    out = nc.dram_tensor("out", [S, D], BF, kind="ExternalOutput")

    with tile.TileContext(nc) as tc:
        _emit(tc, nc, xT, wq, wk, wv, wo, cosT, sinT, rmat, masks, out)
    nc.compile()
    return nc


def _emit(tc, nc, xT, wq, wk, wv, wo, cosT, sinT, rmat, masks, out):
    import contextlib
    ctx = contextlib.ExitStack()
    with ctx:
        const = ctx.enter_context(tc.tile_pool(name="const", bufs=1))
        work = ctx.enter_context(tc.tile_pool(name="work", bufs=1))
        tmp = ctx.enter_context(tc.tile_pool(name="tmp", bufs=4))
        pt_pool = ctx.enter_context(tc.tile_pool(name="pt", bufs=9))
        outp = ctx.enter_context(tc.tile_pool(name="outp", bufs=4))
        ps = ctx.enter_context(tc.tile_pool(name="ps", bufs=8, space="PSUM"))
        dram = ctx.enter_context(tc.tile_pool(name="dramb", bufs=4, space="DRAM"))

        # ---- constants / weights into SBUF ----
        # Emission order == consumption order. wk is split across queues so
        # the very first k-matmul unblocks fast; cos/sin/wv/masks/wo queue
        # behind the x/wq stream (they are consumed later).
        rmat_sb = const.tile([HD, HD], BF, tag="rmat")
        ones_sb = const.tile([128, 1], BF, tag="ones")
        nc.vector.memset(ones_sb[:], 1.0)
        # Leading transfers kept small so the first k/q matmuls unblock
        # fast; later groups are bigger (HWDGE launch overhead is per-DMA).
        wk_sb = const.tile([128, NDC, HD], BF, tag="wk")
        nc.sync.dma_start(out=wk_sb[:, 0:4, :], in_=wk.ap()[:, 0:4, :])
        # chunk groups of [2,2,4,4,...]: first transfers small so the first
        # matmuls unblock early, later ones big to amortize launch overhead
        gx = {}
        gw = {}
        for c in range(0, NDC, 2):
            g = const.tile([128, 2, S], BF, tag=f"xg{c//2}", name=f"xg{c//2}")
            nc.sync.dma_start(out=g[:], in_=xT.ap()[:, c:c + 2, :])
            gx[c], gx[c + 1] = g[:, 0, :], g[:, 1, :]
            if c % 4 == 0:
                g = const.tile([128, 4, QW], BF, tag=f"wqg{c//4}", name=f"wqg{c//4}")
                nc.sync.dma_start(out=g[:], in_=wq.ap()[:, c:c + 4, :])
                for k in range(4):
                    gw[c + k] = g[:, k, :]
            if c == 2:  # rest of wk + rmat behind the first two x/wq groups
                nc.sync.dma_start(out=wk_sb[:, 4:, :], in_=wk.ap()[:, 4:, :])
                nc.sync.dma_start(out=rmat_sb[:], in_=rmat.ap())
        x_sb = [gx[c] for c in range(NDC)]
        wq_sb = [gw[c] for c in range(NDC)]
        cos_sb = const.tile([HD, S], BF, tag="cos")
        nc.sync.dma_start(out=cos_sb[:], in_=cosT.ap())
        sin_sb = const.tile([HD, S], BF, tag="sin")
        nc.sync.dma_start(out=sin_sb[:], in_=sinT.ap())
        wv_sb = const.tile([128, NDC, HD], BF, tag="wv")
        nc.sync.dma_start(out=wv_sb[:], in_=wv.ap())
        mask_sb = const.tile([128, 4, 512], BF, tag="mask")
        nc.sync.dma_start(out=mask_sb[:], in_=masks.ap())
        wo_sb = const.tile([128, HPC, D], BF, tag="wo")
        for n in range(2):
            nc.sync.dma_start(out=wo_sb[:, :, n * 2048:(n + 1) * 2048],
                              in_=wo.ap()[:, :, n * 2048:(n + 1) * 2048])

        # persistent activations
        qhat = {}
        khat = {}
        for j in range(2):
            khat[j] = work.tile([HD, 512], BF, tag=f"khat{j}", name=f"khat{j}")
            for h in range(HPC):
                qhat[(h, j)] = work.tile([HD, 512], BF, tag=f"qhat{h}_{j}", name=f"qhat{h}_{j}")
        v_sb = [work.tile([128, HD], BF, tag=f"v{i}", name=f"v{i}") for i in range(8)]
        ctx_sb = {(h, j): work.tile([HD, 512], BF, tag=f"ctx{h}_{j}", name=f"ctx{h}_{j}")
                  for j in range(2) for h in range(HPC)}

        def rope_copy(src_psum):
            raw = tmp.tile([HD, 512], BF, tag="rope_raw", name="rope_raw", bufs=4)
            nc.scalar.activation(raw[:], src_psum[:],
                                 mybir.ActivationFunctionType.Copy)
            return raw

        def rope_rest(dst, raw, j):
            rq = ps.tile([HD, 512], F32, tag="ps", name="ps")
            nc.tensor.matmul(rq[:], rmat_sb[:], raw[:], start=True, stop=True)
            t1 = tmp.tile([HD, 512], BF, tag="rope_t1", name="rope_t1", bufs=2)
            nc.vector.tensor_mul(t1[:], raw[:], cos_sb[:, j * 512:(j + 1) * 512])
            t2 = tmp.tile([HD, 512], BF, tag="rope_t2", name="rope_t2", bufs=2)
            nc.vector.tensor_mul(t2[:], rq[:], sin_sb[:, j * 512:(j + 1) * 512])
            nc.vector.tensor_add(dst[:], t1[:], t2[:])

        # ---- QKV projections + RoPE (per s-half) ----
        # Chunk-major: the k-chain and all four q-chains advance together
        # per x-chunk, so PE starts as soon as chunk 0 lands and is paced
        # by compute, not by the x/wq DMA stream.
        for j in range(2):
            sl = slice(j * 512, (j + 1) * 512)
            kp = ps.tile([HD, 512], F32, tag="ps", name="ps")
            qps = [ps.tile([HD, 512], F32, tag="ps", name=f"qp{h}")
                   for h in range(HPC)]
            for c in range(NDC):
                nc.tensor.matmul(kp[:], wk_sb[:, c, :], x_sb[c][:, sl],
                                 start=(c == 0), stop=(c == NDC - 1))
                for h in range(HPC):
                    nc.tensor.matmul(qps[h][:], wq_sb[c][:, h * HD:(h + 1) * HD],
                                     x_sb[c][:, sl],
                                     start=(c == 0), stop=(c == NDC - 1))
            rope_rest(khat[j], rope_copy(kp), j)
            for h in range(HPC):
                rope_rest(qhat[(h, j)], rope_copy(qps[h]), j)
        for i in range(8):
            vp = ps.tile([128, HD], F32, tag="ps", name="ps")
            for c in range(NDC):
                nc.tensor.matmul(vp[:], x_sb[c][:, i * 128:(i + 1) * 128],
                                 wv_sb[:, c, :],
                                 start=(c == 0), stop=(c == NDC - 1))
            nc.vector.tensor_copy(v_sb[i][:], vp[:])

        # ---- attention + out-proj, per s-half ----
        for j in range(2):
            for h in range(HPC):
                tt = _t_tiles(j)
                pts = []
                for n, (i, m) in enumerate(tt):
                    kj, ki = divmod(i, 4)
                    st = ps.tile([128, 512], F32, tag="ps", name="ps")
                    nc.tensor.matmul(st[:], khat[kj][:, ki * 128:(ki + 1) * 128],
                                     qhat[(h, j)][:], start=True, stop=True)
                    pt = pt_pool.tile([128, 512], BF, tag="pt", name="pt")
                    if m is None:
                        nc.scalar.activation(pt[:], st[:],
                                             mybir.ActivationFunctionType.Exp)
                    else:
                        et = tmp.tile([128, 512], BF, tag="exp_tmp", name="exp_tmp", bufs=2)
                        nc.scalar.activation(et[:], st[:],
                                             mybir.ActivationFunctionType.Exp)
                        nc.vector.tensor_mul(pt[:], et[:], mask_sb[:, m, :])
                    pts.append((i, pt))
                den = ps.tile([1, 512], F32, tag="ps", name="ps")
                for n, (i, pt) in enumerate(pts):
                    nc.tensor.matmul(den[:], ones_sb[:], pt[:],
                                     start=(n == 0), stop=(n == len(pts) - 1))
                rec = tmp.tile([1, 512], F32, tag="rec", name="rec", bufs=2)
                nc.vector.reciprocal(rec[:], den[:])
                bc = tmp.tile([128, 512], F32, tag="bc", name="bc", bufs=2)
                nc.gpsimd.partition_broadcast(bc[:], rec[:])
                cx = ps.tile([HD, 512], F32, tag="ps", name="ps")
                for n, (i, pt) in enumerate(pts):
                    nc.tensor.matmul(cx[:], v_sb[i][:], pt[:],
                                     start=(n == 0), stop=(n == len(pts) - 1))
                nc.vector.tensor_mul(ctx_sb[(h, j)][:], cx[:], bc[:])

            # out-proj for the 4 token tiles of this half; two 512-wide psum
            # chains share one bf16 [128,1024] staging tile and one DMA
            for q in range(4):
                st_i = 4 * j + q
                for n2 in range(4):
                    ot = outp.tile([128, 1024], BF, tag="ot", name="ot")
                    for sub in range(2):
                        n = 2 * n2 + sub
                        op = ps.tile([128, 512], F32, tag="ps", name="ps")
                        for h in range(HPC):
                            nc.tensor.matmul(
                                op[:],
                                ctx_sb[(h, j)][:, q * 128:(q + 1) * 128],
                                wo_sb[:, h, n * 512:(n + 1) * 512],
                                start=(h == 0), stop=(h == HPC - 1))
                        nc.vector.tensor_copy(ot[:, sub * 512:(sub + 1) * 512],
                                              op[:])
                    nc.sync.dma_start(
                        out=out.ap()[st_i * 128:(st_i + 1) * 128,
                                     n2 * 1024:(n2 + 1) * 1024],
                        in_=ot[:])


def _prep_inputs(x, cos, sin, Wq, Wk, Wv, Wo):
    """Host-side shard + layout prep. Returns per-core input maps."""
    bf = ml_dtypes.bfloat16
    x2 = np.asarray(x, np.float32).reshape(S, D)
    xTh = np.ascontiguousarray(x2.T).reshape(NDC, 128, S).transpose(1, 0, 2)
    xTh = np.ascontiguousarray(xTh).astype(bf)

    cosT = np.ascontiguousarray(np.asarray(cos, np.float32).T).astype(bf)
    sinT = np.ascontiguousarray(np.asarray(sin, np.float32).T).astype(bf)

    rmat = np.zeros((HD, HD), np.float32)
    half = HD // 2
    rmat[np.arange(half), np.arange(half) + half] = 1.0
    rmat[np.arange(half) + half, np.arange(half)] = -1.0
    rmat = rmat.astype(bf)

    lt = np.arange(128)[:, None]
    ls = np.arange(512)[None, :]
    masks = np.stack([(lt + 128 * m <= ls) for m in range(4)], axis=0)
    masks = np.ascontiguousarray(masks.transpose(1, 0, 2)).astype(bf)  # [128,4,512]

    scale = 1.0 / np.sqrt(np.float32(HD))
    Wq = np.asarray(Wq, np.float32) * scale
    Wk = np.asarray(Wk, np.float32)
    Wv = np.asarray(Wv, np.float32)
    Wo = np.asarray(Wo, np.float32)

    def chunked(w):  # [D, m] -> [128, NDC, m]
        m = w.shape[1]
        return np.ascontiguousarray(
            w.reshape(NDC, 128, m).transpose(1, 0, 2)).astype(bf)

    in_maps = []
    for r in range(N_CORES):
        wq_r = chunked(Wq[:, r * QW:(r + 1) * QW])
        wk_r = chunked(Wk[:, r * HD:(r + 1) * HD])
        wv_r = chunked(Wv[:, r * HD:(r + 1) * HD])
        wo_r = np.ascontiguousarray(
            Wo[r * QW:(r + 1) * QW, :].reshape(HPC, 128, D)
            .transpose(1, 0, 2)).astype(bf)
        in_maps.append({
            "xT": xTh, "wq": wq_r, "wk": wk_r, "wv": wv_r, "wo": wo_r,
            "cosT": cosT, "sinT": sinT, "rmat": rmat, "masks": masks,
        })
    return in_maps


def get_nc():
    if "nc" not in _CACHE:
        _CACHE["nc"] = _build()
    return _CACHE["nc"]


def kernel(x, mask, cos, sin, Wq, Wk, Wv, Wo):
    nc = get_nc()
    in_maps = _prep_inputs(x, cos, sin, Wq, Wk, Wv, Wo)
    res = run_bass_kernel_spmd(nc, in_maps, core_ids=list(range(N_CORES)))
    acc = np.zeros((S, D), np.float32)
    for r in range(N_CORES):
        acc += res.results[r]["out"].astype(np.float32)
    return acc[None]


if __name__ == "__main__":
    rng = np.random.default_rng(0)
    xs = rng.standard_normal((1, S, D), dtype=np.float32)
    print("built:", get_nc() is not None)



# revision 6
# speedup vs baseline: 1.0990x; 1.0990x over previous
"""Grouped-query attention, tensor-parallel over heads across 8 TRN2 NeuronCores.

Problem (hardcoded): x[1,1024,4096] @ Wq/Wk/Wv -> RoPE -> causal GQA
(32 q heads, 8 kv groups, head_dim 128) -> out proj Wo -> [1,1024,4096].

Sharding: core r owns q heads 4r..4r+3 and kv group r (Wq/Wk/Wv column
shards, Wo row shard). Each core computes a full [1024,4096] partial of
the output projection; the host sums the 8 partials (the "all-reduce").

Projections run as fp8 hi-lo DoubleRow matmuls: every operand is split
into e4m3 hi + lo parts (combined quantization error ~0.1%, better than
bf16) and each product is computed with three DoubleRow matmuls per pair
of 128-deep contraction chunks (hi*hi, lo*hi, hi*lo; the dropped lo*lo
term is ~0.06%).  DoubleRow contracts 256 rows per pass at half the
per-column cost of bf16, so projections run at 0.75x the bf16 cycle
count.  Attention (scores/softmax/ctx) stays bf16.
"""

import numpy as np
import ml_dtypes

import concourse.bass as bass
import concourse.bacc as bacc
import concourse.mybir as mybir
import concourse.tile as tile
from concourse.bass_utils import run_bass_kernel_spmd

S = 1024          # sequence length
D = 4096          # model dim
H = 32            # query heads (global)
G = 8             # kv groups (global)
HD = 128          # head dim
N_CORES = 8
HPC = H // N_CORES   # 4 query heads per core
NDC = D // 128       # 32 contraction chunks
NCP = NDC // 2       # 16 chunk pairs
BF = mybir.dt.bfloat16
F32 = mybir.dt.float32
E4 = mybir.dt.float8e4
DR = mybir.MatmulPerfMode.DoubleRow

# host-side hi/lo scales (fp8 payloads are SCALE*true value)
SC_WQ = 512.0     # Wq with 1/sqrt(HD) folded  (sigma ~0.0018 -> ~0.9)
SC_WK = 64.0      # Wk/Wv/Wo sigma 0.02 -> ~1.28
SC_QH = 1.0       # qhat stored at true scale in bf16
_CACHE = {}


def _t_tiles(j):
    """Causal t-tile list for the 512-wide s-tile j, with mask index or None."""
    out = []
    for i in range(4 * j + 4):
        lo = i - 4 * j
        out.append((i, lo if 0 <= lo <= 3 else None))
    return out


def _build():
    nc = bacc.Bacc("TRN2", target_bir_lowering=False, debug=False,
                   num_devices=N_CORES)

    # layouts: hilo-major so every DoubleRow pair slice is contiguous
    xq = nc.dram_tensor("xq", [128, 2, 2, NDC, 512], E4, kind="ExternalInput")
    wq = nc.dram_tensor("wq", [128, 2, HPC, NDC, 128], E4, kind="ExternalInput")
    wk = nc.dram_tensor("wk", [128, 2, NDC, 128], E4, kind="ExternalInput")
    wv = nc.dram_tensor("wv", [128, 2, NDC, 128], E4, kind="ExternalInput")
    wo = nc.dram_tensor("wo", [128, 2, 8, HPC, 512], E4, kind="ExternalInput")
    cosT = nc.dram_tensor("cosT", [HD, S], BF, kind="ExternalInput")
    sinT = nc.dram_tensor("sinT", [HD, S], BF, kind="ExternalInput")
    rmat = nc.dram_tensor("rmat", [HD, HD], BF, kind="ExternalInput")
    masks = nc.dram_tensor("masks", [128, 4, 512], BF, kind="ExternalInput")
    out = nc.dram_tensor("out", [S, D], BF, kind="ExternalOutput")

    with tile.TileContext(nc) as tc:
        _emit(tc, nc, xq, wq, wk, wv, wo, cosT, sinT, rmat, masks, out)
    nc.compile()
    return nc


def _emit(tc, nc, xq, wq, wk, wv, wo, cosT, sinT, rmat, masks, out):
    import contextlib
    ctx = contextlib.ExitStack()
    with ctx:
        const = ctx.enter_context(tc.tile_pool(name="const", bufs=1))
        work = ctx.enter_context(tc.tile_pool(name="work", bufs=1))
        tmp = ctx.enter_context(tc.tile_pool(name="tmp", bufs=4))
        pt_pool = ctx.enter_context(tc.tile_pool(name="pt", bufs=4))
        outp = ctx.enter_context(tc.tile_pool(name="outp", bufs=3))
        ps = ctx.enter_context(tc.tile_pool(name="ps", bufs=1, space="PSUM"))

        # ---- constants / weights into SBUF ----
        rmat_sb = const.tile([HD, HD], BF, tag="rmat")
        ones_sb = const.tile([128, 1], BF, tag="ones")
        nc.vector.memset(ones_sb[:], 1.0)
        wk_sb = const.tile([128, 2, NDC, 128], E4, tag="wk")
        nc.sync.dma_start(out=wk_sb[:, :, 0:8, :], in_=wk.ap()[:, :, 0:8, :])
        # x arrives in chunk groups so the first matmuls unblock early;
        # consumption is j-major then chunk-major across both hilo planes.
        x_sb = const.tile([128, 2, 2, NDC, 512], E4, tag="x")
        wq_sb = const.tile([128, 2, HPC, NDC, 128], E4, tag="wq")
        for j in range(2):
            for c0 in (0, 8, 16, 24):
                for hl in range(2):
                    nc.sync.dma_start(
                        out=x_sb[:, hl, j, c0:c0 + 8, :],
                        in_=xq.ap()[:, hl, j, c0:c0 + 8, :])
                if j == 0:
                    for hl in range(2):
                        nc.sync.dma_start(
                            out=wq_sb[:, hl, :, c0:c0 + 8, :],
                            in_=wq.ap()[:, hl, :, c0:c0 + 8, :])
            if j == 0:
                nc.sync.dma_start(out=wk_sb[:, :, 8:, :], in_=wk.ap()[:, :, 8:, :])
                nc.sync.dma_start(out=rmat_sb[:], in_=rmat.ap())
        cos_sb = const.tile([HD, S], BF, tag="cos")
        nc.sync.dma_start(out=cos_sb[:], in_=cosT.ap())
        sin_sb = const.tile([HD, S], BF, tag="sin")
        nc.sync.dma_start(out=sin_sb[:], in_=sinT.ap())
        wv_sb = const.tile([128, 2, NDC, 128], E4, tag="wv")
        nc.sync.dma_start(out=wv_sb[:], in_=wv.ap())
        mask_sb = const.tile([128, 4, 512], BF, tag="mask")
        nc.sync.dma_start(out=mask_sb[:], in_=masks.ap())
        wo_sb = const.tile([128, 2, 8, HPC, 512], E4, tag="wo")
        for hl in range(2):
            nc.sync.dma_start(out=wo_sb[:, hl], in_=wo.ap()[:, hl])

        # persistent activations
        khat = work.tile([HD, 8, 128], BF, tag="khat")       # per t-tile
        qhat = {(h, j): work.tile([HD, 512], BF, tag=f"qh{h}_{j}",
                                  name=f"qh{h}_{j}")
                for h in range(HPC) for j in range(2)}
        v_sb = work.tile([128, 8, HD], BF, tag="v")          # [t, tile, hd]
        # ctx hi/lo fp8: [hd, hilo, q(8), h, 128] - head pairs contiguous
        ctxq = work.tile([HD, 2, 8, HPC, 128], E4, tag="ctxq")

        def mm3(acc, w_of_hl, x_of_hl, first, last):
            """Three hi-lo DoubleRow matmuls (hi*hi, lo*hi, hi*lo)."""
            nc.tensor.matmul(acc, w_of_hl(0), x_of_hl(0),
                             start=first, stop=False, perf_mode=DR)
            nc.tensor.matmul(acc, w_of_hl(1), x_of_hl(0),
                             start=False, stop=False, perf_mode=DR)
            nc.tensor.matmul(acc, w_of_hl(0), x_of_hl(1),
                             start=False, stop=last, perf_mode=DR)

        def rope_rest(dst, raw, j):
            rq = ps.tile([HD, 512], F32, tag="ps", name="ps", bufs=4)
            nc.tensor.matmul(rq[:], rmat_sb[:], raw[:], start=True, stop=True)
            t1 = tmp.tile([HD, 512], BF, tag="rope_t1", name="rope_t1", bufs=2)
            nc.vector.tensor_mul(t1[:], raw[:], cos_sb[:, j * 512:(j + 1) * 512])
            t2 = tmp.tile([HD, 512], BF, tag="rope_t2", name="rope_t2", bufs=2)
            nc.vector.tensor_mul(t2[:], rq[:], sin_sb[:, j * 512:(j + 1) * 512])
            nc.vector.tensor_add(dst, t1[:], t2[:])

        # ---- QKV projections + RoPE (per s-half, chunk-pair-major) ----
        for j in range(2):
            kp = ps.tile([HD, 512], F32, tag="ps", name="kp", bufs=4)
            qp2 = [ps.tile([HD, 1024], F32, tag="ps2", name=f"qp2_{m}", bufs=2)
                   for m in range(2)]
            qps = [qp2[h // 2][:, (h % 2) * 512:(h % 2) * 512 + 512]
                   for h in range(HPC)]
            for cp in range(NCP):
                c = 2 * cp
                first, last = cp == 0, cp == NCP - 1
                xs = lambda hl, c2=c, j2=j: x_sb[:, hl, j2, c2:c2 + 2, :]
                mm3(kp[:], lambda hl, c2=c: wk_sb[:, hl, c2:c2 + 2, :],
                    xs, first, last)
                for h in range(HPC):
                    mm3(qps[h],
                        lambda hl, c2=c, h2=h: wq_sb[:, hl, h2, c2:c2 + 2, :],
                        xs, first, last)
            raw = tmp.tile([HD, 512], BF, tag="rk", name="rk", bufs=2)
            nc.scalar.activation(raw[:], kp[:],
                                 mybir.ActivationFunctionType.Copy,
                                 scale=1.0 / SC_WK)
            rope_rest(khat[:, 4 * j:4 * j + 4, :], raw, j)
            for h in range(HPC):
                rawq = tmp.tile([HD, 512], BF, tag="rq", name="rq", bufs=2)
                nc.scalar.activation(rawq[:], qps[h],
                                     mybir.ActivationFunctionType.Copy,
                                     scale=SC_QH / SC_WQ)
                rope_rest(qhat[(h, j)][:], rawq, j)
            # V for this half's 4 t-tiles (x pair is the strided stationary)
            for ti in range(4):
                i = 4 * j + ti
                sl = slice(ti * 128, (ti + 1) * 128)
                vp = ps.tile([128, HD], F32, tag="ps", name="vp", bufs=4)
                for cp in range(NCP):
                    c = 2 * cp
                    mm3(vp[:],
                        lambda hl, c2=c, j2=j, s2=sl: x_sb[:, hl, j2,
                                                           c2:c2 + 2, s2],
                        lambda hl, c2=c: wv_sb[:, hl, c2:c2 + 2, :],
                        cp == 0, cp == NCP - 1)
                nc.vector.tensor_scalar_mul(v_sb[:, i, :], vp[:], 1.0 / SC_WK)

        # ---- attention + out-proj, per s-half ----
        for j in range(2):
            for h in range(HPC):
                tt = _t_tiles(j)
                pts = []
                # t-tile pairs share a 2-bank psum + one batched exp
                for n2 in range((len(tt) + 1) // 2):
                    pair = tt[2 * n2:2 * n2 + 2]
                    st = ps.tile([128, 1024], F32, tag="ps2", name="st", bufs=2)
                    for sub, (i, m) in enumerate(pair):
                        nc.tensor.matmul(st[:, sub * 512:(sub + 1) * 512],
                                         khat[:, i, :], qhat[(h, j)][:],
                                         start=True, stop=True)
                    pt = pt_pool.tile([128, 1024], BF, tag="pt", name="pt")
                    nc.scalar.activation(pt[:], st[:],
                                         mybir.ActivationFunctionType.Exp)
                    for sub, (i, m) in enumerate(pair):
                        if m is not None:
                            ss = slice(sub * 512, (sub + 1) * 512)
                            nc.vector.tensor_mul(pt[:, ss], pt[:, ss],
                                                 mask_sb[:, m, :])
                    for sub, (i, m) in enumerate(pair):
                        pts.append((i, pt, slice(sub * 512, (sub + 1) * 512)))
                den = ps.tile([1, 512], F32, tag="ps", name="den", bufs=4)
                for n, (i, pt, ss) in enumerate(pts):
                    nc.tensor.matmul(den[:], ones_sb[:], pt[:, ss],
                                     start=(n == 0), stop=(n == len(pts) - 1))
                rec = tmp.tile([1, 512], F32, tag="rec", name="rec", bufs=2)
                nc.vector.reciprocal(rec[:], den[:])
                bc = tmp.tile([128, 512], F32, tag="bc", name="bc", bufs=2)
                nc.gpsimd.partition_broadcast(bc[:], rec[:])
                cx = ps.tile([HD, 512], F32, tag="ps", name="cx", bufs=4)
                for n, (i, pt, ss) in enumerate(pts):
                    nc.tensor.matmul(cx[:], v_sb[:, i, :], pt[:, ss],
                                     start=(n == 0), stop=(n == len(pts) - 1))
                ctxf = tmp.tile([HD, 512], F32, tag="cxf", name="cxf", bufs=2)
                nc.vector.tensor_mul(ctxf[:], cx[:], bc[:])
                # hi/lo split into ctxq[:, hl, 4j..4j+3, h, :]
                dst_h = ctxq[:, 0, 4 * j:4 * j + 4, h, :]
                dst_l = ctxq[:, 1, 4 * j:4 * j + 4, h, :]
                nc.scalar.activation(dst_h, ctxf[:],
                                     mybir.ActivationFunctionType.Copy)
                nc.vector.tensor_sub(dst_l, ctxf[:], dst_h)

            # out-proj for this half's 4 token tiles
            for q in range(4):
                st_i = 4 * j + q
                for n2 in range(4):
                    op = ps.tile([128, 1024], F32, tag="ps2", name="op", bufs=2)
                    for sub in range(2):
                        n = 2 * n2 + sub
                        dst = op[:, sub * 512:(sub + 1) * 512]
                        for mp in range(2):
                            hs = slice(2 * mp, 2 * mp + 2)
                            mm3(dst,
                                lambda hl, q2=st_i, h2=hs: ctxq[:, hl, q2, h2, :],
                                lambda hl, n3=n, h2=hs: wo_sb[:, hl, n3, h2, :],
                                mp == 0, mp == 1)
                    ot = outp.tile([128, 1024], BF, tag="ot", name="ot")
                    if n2 % 2 == 0:
                        nc.vector.tensor_scalar_mul(ot[:], op[:], 1.0 / SC_WK)
                    else:
                        nc.scalar.activation(ot[:], op[:],
                                             mybir.ActivationFunctionType.Copy,
                                             scale=1.0 / SC_WK)
                    nc.sync.dma_start(
                        out=out.ap()[st_i * 128:(st_i + 1) * 128,
                                     n2 * 1024:(n2 + 1) * 1024],
                        in_=ot[:])


def _hilo(a):
    """e4m3 hi+lo decomposition (fp32 in, two e4m3 out)."""
    e4np = ml_dtypes.float8_e4m3
    hi = np.clip(a, -240, 240).astype(e4np)
    lo = np.clip(a - hi.astype(np.float32), -240, 240).astype(e4np)
    return hi, lo


def _prep_inputs(x, cos, sin, Wq, Wk, Wv, Wo):
    """Host-side shard + layout prep. Returns per-core input maps."""
    bf = ml_dtypes.bfloat16
    x2 = np.asarray(x, np.float32).reshape(S, D)

    # x: [128, hilo, j, c, s]
    xT = np.ascontiguousarray(x2.T)                      # [D, S]
    xh, xl = _hilo(xT)
    xq = np.stack([xh, xl], axis=0).reshape(2, NDC, 128, 2, 512)
    xq = np.ascontiguousarray(xq.transpose(2, 0, 3, 1, 4))  # [128,2,2,NDC,512]

    cosT = np.ascontiguousarray(np.asarray(cos, np.float32).T).astype(bf)
    sinT = np.ascontiguousarray(np.asarray(sin, np.float32).T).astype(bf)

    rmat = np.zeros((HD, HD), np.float32)
    half = HD // 2
    rmat[np.arange(half), np.arange(half) + half] = 1.0
    rmat[np.arange(half) + half, np.arange(half)] = -1.0
    rmat = rmat.astype(bf)

    lt = np.arange(128)[:, None]
    ls = np.arange(512)[None, :]
    masks = np.stack([(lt + 128 * m <= ls) for m in range(4)], axis=0)
    masks = np.ascontiguousarray(masks.transpose(1, 0, 2)).astype(bf)

    scale = 1.0 / np.sqrt(np.float32(HD))
    Wq = np.asarray(Wq, np.float32) * (scale * SC_WQ)
    Wk = np.asarray(Wk, np.float32) * SC_WK
    Wv = np.asarray(Wv, np.float32) * SC_WK
    Wo = np.asarray(Wo, np.float32) * SC_WK

    in_maps = []
    for r in range(N_CORES):
        # wq: [128, hilo, h, c, col]
        wq_r = Wq[:, r * HPC * HD:(r + 1) * HPC * HD]     # [D, 512]
        h_, l_ = _hilo(wq_r)
        wq_p = np.stack([h_, l_], 0).reshape(2, NDC, 128, HPC, 128)
        wq_p = np.ascontiguousarray(wq_p.transpose(2, 0, 3, 1, 4))

        def kv_pack(w):                                   # [D,128]->[128,2,NDC,128]
            h2, l2 = _hilo(w)
            p = np.stack([h2, l2], 0).reshape(2, NDC, 128, 128)
            return np.ascontiguousarray(p.transpose(2, 0, 1, 3))

        wk_p = kv_pack(Wk[:, r * HD:(r + 1) * HD])
        wv_p = kv_pack(Wv[:, r * HD:(r + 1) * HD])

        # wo: [128, hilo, n, h, col]; rows r*512..(r+1)*512 of Wo
        wo_r = Wo[r * HPC * HD:(r + 1) * HPC * HD, :]     # [512, D]
        h_, l_ = _hilo(wo_r)
        wo_p = np.stack([h_, l_], 0).reshape(2, HPC, 128, 8, 512)
        wo_p = np.ascontiguousarray(wo_p.transpose(2, 0, 3, 1, 4))

        in_maps.append({
            "xq": xq, "wq": wq_p, "wk": wk_p, "wv": wv_p, "wo": wo_p,
            "cosT": cosT, "sinT": sinT, "rmat": rmat, "masks": masks,
        })
    return in_maps


def get_nc():
    if "nc" not in _CACHE:
        _CACHE["nc"] = _build()
    return _CACHE["nc"]


def kernel(x, mask, cos, sin, Wq, Wk, Wv, Wo):
    nc = get_nc()
    in_maps = _prep_inputs(x, cos, sin, Wq, Wk, Wv, Wo)
    res = run_bass_kernel_spmd(nc, in_maps, core_ids=list(range(N_CORES)))
    acc = np.zeros((S, D), np.float32)
    for r in range(N_CORES):
        acc += res.results[r]["out"].astype(np.float32)
    return acc[None]


if __name__ == "__main__":
    print("built:", get_nc() is not None)


# revision 13
# speedup vs baseline: 1.1758x; 1.0700x over previous
"""Grouped-query attention, tensor-parallel over heads across 8 TRN2 NeuronCores.

Problem (hardcoded): x[1,1024,4096] @ Wq/Wk/Wv -> RoPE -> causal GQA
(32 q heads, 8 kv groups, head_dim 128) -> out proj Wo -> [1,1024,4096].

Sharding: core r owns q heads 4r..4r+3 and kv group r (Wq/Wk/Wv column
shards, Wo row shard). Each core computes a full [1024,4096] partial of
the output projection; the host sums the 8 partials (the "all-reduce").

Projections run as fp8 hi-lo DoubleRow matmuls: every operand is split
into e4m3 hi + lo parts (combined quantization error ~0.1%, better than
bf16) and each product is computed with three DoubleRow matmuls per pair
of 128-deep contraction chunks (hi*hi, lo*hi, hi*lo; the dropped lo*lo
term is ~0.06%).  DoubleRow contracts 256 rows per pass at half the
per-column cost of bf16, so projections run at 0.75x the bf16 cycle
count.  Attention (scores/softmax/ctx) stays bf16.
"""

import numpy as np
import ml_dtypes

import concourse.bass as bass
import concourse.bacc as bacc
import concourse.mybir as mybir
import concourse.tile as tile
from concourse.bass_utils import run_bass_kernel_spmd

S = 1024          # sequence length
D = 4096          # model dim
H = 32            # query heads (global)
G = 8             # kv groups (global)
HD = 128          # head dim
N_CORES = 8
HPC = H // N_CORES   # 4 query heads per core
NDC = D // 128       # 32 contraction chunks
NCP = NDC // 2       # 16 chunk pairs
BF = mybir.dt.bfloat16
F32 = mybir.dt.float32
E4 = mybir.dt.float8e4
DR = mybir.MatmulPerfMode.DoubleRow

# host-side hi/lo scales (fp8 payloads are SCALE*true value)
SC_WQ = 512.0     # Wq with 1/sqrt(HD) folded  (sigma ~0.0018 -> ~0.9)
SC_WK = 64.0      # Wk/Wv/Wo sigma 0.02 -> ~1.28
SC_QH = 1.0       # qhat stored at true scale in bf16
_CACHE = {}


def _t_tiles(j):
    """Causal t-tile list for the 512-wide s-tile j, with mask index or None."""
    out = []
    for i in range(4 * j + 4):
        lo = i - 4 * j
        out.append((i, lo if 0 <= lo <= 3 else None))
    return out


def _build():
    nc = bacc.Bacc("TRN2", target_bir_lowering=False, debug=False,
                   num_devices=N_CORES)

    # layouts: hilo-major so every DoubleRow pair slice is contiguous
    xq = nc.dram_tensor("xq", [128, 2, 2, NDC, 512], E4, kind="ExternalInput")
    wq = nc.dram_tensor("wq", [128, 2, HPC, NDC, 128], E4, kind="ExternalInput")
    wk = nc.dram_tensor("wk", [128, 2, NDC, 128], E4, kind="ExternalInput")
    wv = nc.dram_tensor("wv", [128, 2, NDC, 128], E4, kind="ExternalInput")
    wo = nc.dram_tensor("wo", [128, 2, 8, HPC, 512], E4, kind="ExternalInput")
    cosT = nc.dram_tensor("cosT", [HD, S], BF, kind="ExternalInput")
    sinT = nc.dram_tensor("sinT", [HD, S], BF, kind="ExternalInput")
    rmat = nc.dram_tensor("rmat", [HD, HD], BF, kind="ExternalInput")
    masks = nc.dram_tensor("masks", [128, 4, 512], BF, kind="ExternalInput")
    out = nc.dram_tensor("out", [S, D], BF, kind="ExternalOutput")

    with tile.TileContext(nc) as tc:
        _emit(tc, nc, xq, wq, wk, wv, wo, cosT, sinT, rmat, masks, out)
    nc.compile()
    return nc


def _emit(tc, nc, xq, wq, wk, wv, wo, cosT, sinT, rmat, masks, out):
    import contextlib
    ctx = contextlib.ExitStack()
    with ctx:
        const = ctx.enter_context(tc.tile_pool(name="const", bufs=1))
        work = ctx.enter_context(tc.tile_pool(name="work", bufs=1))
        tmp = ctx.enter_context(tc.tile_pool(name="tmp", bufs=4))
        pt_pool = ctx.enter_context(tc.tile_pool(name="pt", bufs=4))
        outp = ctx.enter_context(tc.tile_pool(name="outp", bufs=4))
        ps = ctx.enter_context(tc.tile_pool(name="ps", bufs=1, space="PSUM"))

        # ---- constants / weights into SBUF ----
        # DMA priority stream: wk, then x-hi(j0)‖wq-hi groups, wq-lo,
        # x-lo(j0), cos/sin/wv, x(j1), masks, wo.  The QKV j0 product runs
        # as three passes (hi*hi, lo*hi, hi*lo) so compute starts on x-hi
        # alone and each pass is fed just-in-time.
        rmat_sb = const.tile([HD, HD], BF, tag="rmat")
        ones_sb = const.tile([128, 1], BF, tag="ones")
        nc.vector.memset(ones_sb[:], 1.0)
        wk_sb = const.tile([128, 2, NDC, 128], E4, tag="wk")
        nc.sync.dma_start(out=wk_sb[:, :, 0:8, :], in_=wk.ap()[:, :, 0:8, :])
        x_sb = const.tile([128, 2, 2, NDC, 512], E4, tag="x")
        wq_sb = const.tile([128, 2, HPC, NDC, 128], E4, tag="wq")
        for c0, c1 in ((0, 4), (4, 8), (8, 16), (16, 24)):
            nc.sync.dma_start(out=x_sb[:, 0, 0, c0:c1, :],
                              in_=xq.ap()[:, 0, 0, c0:c1, :])
            nc.sync.dma_start(out=wq_sb[:, 0, :, c0:c1, :],
                              in_=wq.ap()[:, 0, :, c0:c1, :])
        nc.sync.dma_start(out=wq_sb[:, 1, :, 0:8, :],
                          in_=wq.ap()[:, 1, :, 0:8, :])
        nc.sync.dma_start(out=x_sb[:, 0, 0, 24:32, :],
                          in_=xq.ap()[:, 0, 0, 24:32, :])
        nc.sync.dma_start(out=wq_sb[:, 0, :, 24:32, :],
                          in_=wq.ap()[:, 0, :, 24:32, :])
        nc.sync.dma_start(out=wq_sb[:, 1, :, 8:16, :],
                          in_=wq.ap()[:, 1, :, 8:16, :])
        nc.sync.dma_start(out=wk_sb[:, :, 8:, :], in_=wk.ap()[:, :, 8:, :])
        nc.sync.dma_start(out=wq_sb[:, 1, :, 16:32, :],
                          in_=wq.ap()[:, 1, :, 16:32, :])
        nc.sync.dma_start(out=rmat_sb[:], in_=rmat.ap())
        for c0, c1 in ((0, 16), (16, 32)):
            nc.sync.dma_start(out=x_sb[:, 1, 0, c0:c1, :],
                              in_=xq.ap()[:, 1, 0, c0:c1, :])
        cos_sb = const.tile([HD, S], BF, tag="cos")
        nc.sync.dma_start(out=cos_sb[:], in_=cosT.ap())
        sin_sb = const.tile([HD, S], BF, tag="sin")
        nc.sync.dma_start(out=sin_sb[:], in_=sinT.ap())
        wv_sb = const.tile([128, 2, NDC, 128], E4, tag="wv")
        nc.sync.dma_start(out=wv_sb[:], in_=wv.ap())
        for hl in range(2):
            for c0, c1 in ((0, 16), (16, 32)):
                nc.sync.dma_start(out=x_sb[:, hl, 1, c0:c1, :],
                                  in_=xq.ap()[:, hl, 1, c0:c1, :])
        mask_sb = const.tile([128, 4, 512], BF, tag="mask")
        nc.sync.dma_start(out=mask_sb[:], in_=masks.ap())
        wo_sb = const.tile([128, 2, 8, HPC, 512], E4, tag="wo")
        for hl in range(2):
            nc.sync.dma_start(out=wo_sb[:, hl], in_=wo.ap()[:, hl])

        # persistent activations
        khat = work.tile([HD, 8, 128], BF, tag="khat")       # per t-tile
        qhat = {(h, j): work.tile([HD, 512], BF, tag=f"qh{h}_{j}",
                                  name=f"qh{h}_{j}")
                for h in range(HPC) for j in range(2)}
        v_sb = work.tile([128, 8, HD], BF, tag="v")          # [t, tile, hd]
        # ctx hi/lo fp8: [hd, hilo, q(8), h, 128] - head pairs contiguous
        ctxq = work.tile([HD, 2, 8, HPC, 128], E4, tag="ctxq")

        def mmt(acc, w_ap, x_ap, first=False, last=False):
            nc.tensor.matmul(acc, w_ap, x_ap, start=first, stop=last,
                             perf_mode=DR)

        def rope_rest(dst, raw, j):
            rq = ps.tile([HD, 512], F32, tag="ps", name="ps", bufs=4)
            nc.tensor.matmul(rq[:], rmat_sb[:], raw[:], start=True, stop=True)
            t1 = tmp.tile([HD, 512], BF, tag="rope_t1", name="rope_t1", bufs=2)
            nc.vector.tensor_mul(t1[:], raw[:], cos_sb[:, j * 512:(j + 1) * 512])
            t2 = tmp.tile([HD, 512], BF, tag="rope_t2", name="rope_t2", bufs=2)
            nc.vector.tensor_mul(t2[:], rq[:], sin_sb[:, j * 512:(j + 1) * 512])
            nc.vector.tensor_add(dst, t1[:], t2[:])

        def v_chain(j, ti):
            i = 4 * j + ti
            sl = slice(ti * 128, (ti + 1) * 128)
            vp = ps.tile([128, HD], F32, tag="ps", name="vp", bufs=4)
            for cp in range(NCP):
                c = 2 * cp
                xh = x_sb[:, 0, j, c:c + 2, sl]
                xl = x_sb[:, 1, j, c:c + 2, sl]
                mmt(vp[:], xh, wv_sb[:, 0, c:c + 2, :], first=(cp == 0))
                mmt(vp[:], xl, wv_sb[:, 0, c:c + 2, :])
                mmt(vp[:], xh, wv_sb[:, 1, c:c + 2, :], last=(cp == NCP - 1))
            nc.vector.tensor_scalar_mul(v_sb[:, i, :], vp[:], 1.0 / SC_WK)

        # ---- QKV projections + RoPE ----
        def qkv_phase(j):
            kp = ps.tile([HD, 512], F32, tag="ps", name="kp", bufs=4)
            qp2 = [ps.tile([HD, 1024], F32, tag="ps2", name=f"qp2_{m}", bufs=2)
                   for m in range(2)]
            qps = [qp2[h // 2][:, (h % 2) * 512:(h % 2) * 512 + 512]
                   for h in range(HPC)]
            # pass 1: Whi x Xhi   pass 2: Wlo x Xhi   pass 3: Whi x Xlo
            for whl, xhl, first, last in ((0, 0, True, False),
                                          (1, 0, False, False),
                                          (0, 1, False, True)):
                for cp in range(NCP):
                    c = 2 * cp
                    f = first and cp == 0
                    l = last and cp == NCP - 1
                    xap = x_sb[:, xhl, j, c:c + 2, :]
                    mmt(kp[:], wk_sb[:, whl, c:c + 2, :], xap, f, l)
                    for h in range(HPC):
                        mmt(qps[h], wq_sb[:, whl, h, c:c + 2, :], xap, f, l)
            # rope chains interleaved with this half's V chains; raw copies
            # alternate DVE/ACT so the PE rope matmuls aren't gated on one
            # engine's queue
            raw = tmp.tile([HD, 512], BF, tag="rk", name="rk", bufs=2)
            nc.vector.tensor_scalar_mul(raw[:], kp[:], 1.0 / SC_WK)
            v_chain(j, 0)
            rope_rest(khat[:, 4 * j:4 * j + 4, :], raw, j)
            for h in range(HPC):
                rawq = tmp.tile([HD, 512], BF, tag="rq", name="rq", bufs=2)
                if h % 2 == 0:
                    nc.scalar.activation(rawq[:], qps[h],
                                         mybir.ActivationFunctionType.Copy,
                                         scale=SC_QH / SC_WQ)
                else:
                    nc.vector.tensor_scalar_mul(rawq[:], qps[h],
                                                SC_QH / SC_WQ)
                if h < 3:
                    v_chain(j, h + 1)
                rope_rest(qhat[(h, j)][:], rawq, j)

        # ---- out-proj helpers ----
        ot_half = {}

        def op_chain(st_i, n, eng):
            """512-wide out-proj chain + staged copy; DMA per 1024 block."""
            opp = ps.tile([128, 512], F32, tag="ps", name="opp", bufs=4)
            for mp in range(2):
                hs = slice(2 * mp, 2 * mp + 2)
                mmt(opp[:], ctxq[:, 0, st_i, hs, :], wo_sb[:, 0, n, hs, :],
                    first=(mp == 0))
                mmt(opp[:], ctxq[:, 1, st_i, hs, :], wo_sb[:, 0, n, hs, :])
                mmt(opp[:], ctxq[:, 0, st_i, hs, :], wo_sb[:, 1, n, hs, :],
                    last=(mp == 1))
            n2, sub = divmod(n, 2)
            if sub == 0:
                ot_half[(st_i, n2)] = outp.tile([128, 1024], BF, tag="ot",
                                                name="ot")
            ot = ot_half[(st_i, n2)]
            dst = ot[:, sub * 512:(sub + 1) * 512]
            if eng == 0:
                nc.vector.tensor_scalar_mul(dst, opp[:], 1.0 / SC_WK)
            else:
                nc.scalar.activation(dst, opp[:],
                                     mybir.ActivationFunctionType.Copy,
                                     scale=1.0 / SC_WK)
            if sub == 1:
                nc.sync.dma_start(
                    out=out.ap()[st_i * 128:(st_i + 1) * 128,
                                 n2 * 1024:(n2 + 1) * 1024],
                    in_=ot[:])

        # ---- attention, software-pipelined, with PE fill hooks ----
        def attn_phase(j, take):
            """Attention for s-half j.  take(k) emits up to k queued filler
            PE chains.  j=0 heads (2 pairs) pipeline den/cx one head behind
            the scores; j=1 heads run scores 2 pairs ahead."""
            tt = _t_tiles(j)
            npair = len(tt) // 2
            ntile = len(tt)
            pend = []

            def scores(h, p, pts):
                pair = tt[2 * p:2 * p + 2]
                st = ps.tile([128, 1024], F32, tag="ps2", name="st", bufs=2)
                for sub, (i, m) in enumerate(pair):
                    nc.tensor.matmul(st[:, sub * 512:(sub + 1) * 512],
                                     khat[:, i, :], qhat[(h, j)][:],
                                     start=True, stop=True)
                pt = pt_pool.tile([128, 1024], BF, tag="pt", name="pt")
                nc.scalar.activation(pt[:], st[:],
                                     mybir.ActivationFunctionType.Exp)
                for sub, (i, m) in enumerate(pair):
                    if m is not None:
                        ss = slice(sub * 512, (sub + 1) * 512)
                        nc.vector.tensor_mul(pt[:, ss], pt[:, ss],
                                             mask_sb[:, m, :])
                pts[p] = pt

            def dencx(h, pts, interleave):
                den = ps.tile([1, 512], F32, tag="ps", name="den", bufs=4)
                cx = ps.tile([HD, 512], F32, tag="ps", name="cx", bufs=4)
                for p in range(npair):
                    if interleave and p + 2 < npair:
                        scores(h, p + 2, pts)
                    for sub in range(2):
                        n = 2 * p + sub
                        ss = slice(sub * 512, (sub + 1) * 512)
                        nc.tensor.matmul(den[:], ones_sb[:], pts[p][:, ss],
                                         start=(n == 0), stop=(n == ntile - 1))
                    for sub in range(2):
                        n = 2 * p + sub
                        i = tt[n][0]
                        ss = slice(sub * 512, (sub + 1) * 512)
                        nc.tensor.matmul(cx[:], v_sb[:, i, :], pts[p][:, ss],
                                         start=(n == 0), stop=(n == ntile - 1))
                    take(2)
                finish_head(j, h, den, cx)

            for h in range(HPC):
                pts = {}
                scores(h, 0, pts)
                if npair > 1:
                    scores(h, 1, pts)
                if npair == 2:
                    pend.append((h, pts))
                    if len(pend) >= 2:
                        h2, pts2 = pend.pop(0)
                        dencx(h2, pts2, False)
                else:
                    if h > 0:
                        take(1)
                    dencx(h, pts, True)
            for h2, pts2 in pend:
                dencx(h2, pts2, False)

        def finish_head(j, h, den, cx):
            rec = tmp.tile([1, 512], F32, tag="rec", name="rec", bufs=1)
            nc.vector.reciprocal(rec[:], den[:])
            bc = tmp.tile([128, 512], F32, tag="bc", name="bc", bufs=2)
            nc.gpsimd.partition_broadcast(bc[:], rec[:])
            ctxf = tmp.tile([HD, 512], F32, tag="cxf", name="cxf", bufs=2)
            nc.vector.tensor_mul(ctxf[:], cx[:], bc[:])
            dst_h = ctxq[:, 0, 4 * j:4 * j + 4, h, :]
            dst_l = ctxq[:, 1, 4 * j:4 * j + 4, h, :]
            nc.scalar.activation(dst_h, ctxf[:],
                                 mybir.ActivationFunctionType.Copy)
            nc.vector.tensor_sub(dst_l, ctxf[:], dst_h)

        # ---- emission schedule ----
        qkv_phase(0)
        qkv_phase(1)
        attn_phase(0, lambda k: None)
        # attn j1: fill PE waits with 28 of the 32 out-proj(j0) chains; the
        # last 4 cover the attn -> out-proj-j1 boundary
        def eng_of(n):
            return 1 if n % 4 == 3 else 0
        opq = [(s, n) for s in range(4) for n in range(8)]

        def take(k):
            for _ in range(min(k, len(opq))):
                s, n = opq.pop(0)
                op_chain(s, n, eng_of(n))
        attn_phase(1, take)
        take(len(opq))
        # out-proj j1 (dedicated phase, 1024-wide psum chains); the first two
        # tiles' head-pair-0 half chains are emitted up front - they depend
        # only on j1 heads 0/1 and bridge the wait for head 3's ctx
        pre_ops = {}
        for n2 in range(2):
            op = ps.tile([128, 1024], F32, tag="ps2", name="op", bufs=2)
            pre_ops[n2] = op
            for sub in range(2):
                n = 2 * n2 + sub
                dst = op[:, sub * 512:(sub + 1) * 512]
                hs = slice(0, 2)
                mmt(dst, ctxq[:, 0, 4, hs, :], wo_sb[:, 0, n, hs, :],
                    first=True)
                mmt(dst, ctxq[:, 1, 4, hs, :], wo_sb[:, 0, n, hs, :])
                mmt(dst, ctxq[:, 0, 4, hs, :], wo_sb[:, 1, n, hs, :])
        for q in range(4):
            st_i = 4 + q
            for n2 in range(4):
                op = pre_ops.pop(n2, None) if q == 0 else None
                mps = (1,) if op is not None else (0, 1)
                if op is None:
                    op = ps.tile([128, 1024], F32, tag="ps2", name="op",
                                 bufs=2)
                for sub in range(2):
                    n = 2 * n2 + sub
                    dst = op[:, sub * 512:(sub + 1) * 512]
                    for mp in mps:
                        hs = slice(2 * mp, 2 * mp + 2)
                        mmt(dst, ctxq[:, 0, st_i, hs, :],
                            wo_sb[:, 0, n, hs, :], first=(mp == 0))
                        mmt(dst, ctxq[:, 1, st_i, hs, :],
                            wo_sb[:, 0, n, hs, :])
                        mmt(dst, ctxq[:, 0, st_i, hs, :],
                            wo_sb[:, 1, n, hs, :], last=(mp == 1))
                ot = outp.tile([128, 1024], BF, tag="ot", name="ot")
                if q == 3 and n2 == 2:
                    nc.vector.tensor_scalar_mul(ot[:, 0:512], op[:, 0:512],
                                                1.0 / SC_WK)
                    nc.scalar.activation(ot[:, 512:1024], op[:, 512:1024],
                                         mybir.ActivationFunctionType.Copy,
                                         scale=1.0 / SC_WK)
                elif q == 3 and n2 == 3:
                    # final tile: copy+ship halves independently on both
                    # engines so the tail is one 512-block deep
                    nc.vector.tensor_scalar_mul(ot[:, 0:512], op[:, 0:512],
                                                1.0 / SC_WK)
                    nc.sync.dma_start(
                        out=out.ap()[st_i * 128:(st_i + 1) * 128,
                                     n2 * 1024:n2 * 1024 + 512],
                        in_=ot[:, 0:512])
                    nc.scalar.activation(ot[:, 512:1024], op[:, 512:1024],
                                         mybir.ActivationFunctionType.Copy,
                                         scale=1.0 / SC_WK)
                    nc.sync.dma_start(
                        out=out.ap()[st_i * 128:(st_i + 1) * 128,
                                     n2 * 1024 + 512:(n2 + 1) * 1024],
                        in_=ot[:, 512:1024])
                    continue
                elif n2 % 2 == 0:
                    nc.vector.tensor_scalar_mul(ot[:], op[:], 1.0 / SC_WK)
                else:
                    nc.scalar.activation(ot[:], op[:],
                                         mybir.ActivationFunctionType.Copy,
                                         scale=1.0 / SC_WK)
                nc.sync.dma_start(
                    out=out.ap()[st_i * 128:(st_i + 1) * 128,
                                 n2 * 1024:(n2 + 1) * 1024],
                    in_=ot[:])


def _hilo(a):
    """e4m3 hi+lo decomposition (fp32 in, two e4m3 out)."""
    e4np = ml_dtypes.float8_e4m3
    hi = np.clip(a, -240, 240).astype(e4np)
    lo = np.clip(a - hi.astype(np.float32), -240, 240).astype(e4np)
    return hi, lo


def _prep_inputs(x, cos, sin, Wq, Wk, Wv, Wo):
    """Host-side shard + layout prep. Returns per-core input maps."""
    bf = ml_dtypes.bfloat16
    x2 = np.asarray(x, np.float32).reshape(S, D)

    # x: [128, hilo, j, c, s]
    xT = np.ascontiguousarray(x2.T)                      # [D, S]
    xh, xl = _hilo(xT)
    xq = np.stack([xh, xl], axis=0).reshape(2, NDC, 128, 2, 512)
    xq = np.ascontiguousarray(xq.transpose(2, 0, 3, 1, 4))  # [128,2,2,NDC,512]

    cosT = np.ascontiguousarray(np.asarray(cos, np.float32).T).astype(bf)
    sinT = np.ascontiguousarray(np.asarray(sin, np.float32).T).astype(bf)

    rmat = np.zeros((HD, HD), np.float32)
    half = HD // 2
    rmat[np.arange(half), np.arange(half) + half] = 1.0
    rmat[np.arange(half) + half, np.arange(half)] = -1.0
    rmat = rmat.astype(bf)

    lt = np.arange(128)[:, None]
    ls = np.arange(512)[None, :]
    masks = np.stack([(lt + 128 * m <= ls) for m in range(4)], axis=0)
    masks = np.ascontiguousarray(masks.transpose(1, 0, 2)).astype(bf)

    scale = 1.0 / np.sqrt(np.float32(HD))
    Wq = np.asarray(Wq, np.float32) * (scale * SC_WQ)
    Wk = np.asarray(Wk, np.float32) * SC_WK
    Wv = np.asarray(Wv, np.float32) * SC_WK
    Wo = np.asarray(Wo, np.float32) * SC_WK

    in_maps = []
    for r in range(N_CORES):
        # wq: [128, hilo, h, c, col]
        wq_r = Wq[:, r * HPC * HD:(r + 1) * HPC * HD]     # [D, 512]
        h_, l_ = _hilo(wq_r)
        wq_p = np.stack([h_, l_], 0).reshape(2, NDC, 128, HPC, 128)
        wq_p = np.ascontiguousarray(wq_p.transpose(2, 0, 3, 1, 4))

        def kv_pack(w):                                   # [D,128]->[128,2,NDC,128]
            h2, l2 = _hilo(w)
            p = np.stack([h2, l2], 0).reshape(2, NDC, 128, 128)
            return np.ascontiguousarray(p.transpose(2, 0, 1, 3))

        wk_p = kv_pack(Wk[:, r * HD:(r + 1) * HD])
        wv_p = kv_pack(Wv[:, r * HD:(r + 1) * HD])

        # wo: [128, hilo, n, h, col]; rows r*512..(r+1)*512 of Wo
        wo_r = Wo[r * HPC * HD:(r + 1) * HPC * HD, :]     # [512, D]
        h_, l_ = _hilo(wo_r)
        wo_p = np.stack([h_, l_], 0).reshape(2, HPC, 128, 8, 512)
        wo_p = np.ascontiguousarray(wo_p.transpose(2, 0, 3, 1, 4))

        in_maps.append({
            "xq": xq, "wq": wq_p, "wk": wk_p, "wv": wv_p, "wo": wo_p,
            "cosT": cosT, "sinT": sinT, "rmat": rmat, "masks": masks,
        })
    return in_maps


def get_nc():
    if "nc" not in _CACHE:
        _CACHE["nc"] = _build()
    return _CACHE["nc"]


def kernel(x, mask, cos, sin, Wq, Wk, Wv, Wo):
    nc = get_nc()
    in_maps = _prep_inputs(x, cos, sin, Wq, Wk, Wv, Wo)
    res = run_bass_kernel_spmd(nc, in_maps, core_ids=list(range(N_CORES)))
    acc = np.zeros((S, D), np.float32)
    for r in range(N_CORES):
        acc += res.results[r]["out"].astype(np.float32)
    return acc[None]


if __name__ == "__main__":
    print("built:", get_nc() is not None)


# revision 17
# speedup vs baseline: 1.2037x; 1.0237x over previous
"""Grouped-query attention, tensor-parallel over heads across 8 TRN2 NeuronCores.

Problem (hardcoded): x[1,1024,4096] @ Wq/Wk/Wv -> RoPE -> causal GQA
(32 q heads, 8 kv groups, head_dim 128) -> out proj Wo -> [1,1024,4096].

Sharding: core r owns q heads 4r..4r+3 and kv group r (Wq/Wk/Wv column
shards, Wo row shard). Each core computes a full [1024,4096] partial of
the output projection; the host sums the 8 partials (the "all-reduce").

Projections run as fp8 hi-lo DoubleRow matmuls: every operand is split
into e4m3 hi + lo parts (combined quantization error ~0.1%, better than
bf16) and each product is computed with three DoubleRow matmuls per pair
of 128-deep contraction chunks (hi*hi, lo*hi, hi*lo; the dropped lo*lo
term is ~0.06%).  DoubleRow contracts 256 rows per pass at half the
per-column cost of bf16, so projections run at 0.75x the bf16 cycle
count.  Attention (scores/softmax/ctx) stays bf16.
"""

import numpy as np
import ml_dtypes

import concourse.bass as bass
import concourse.bacc as bacc
import concourse.mybir as mybir
import concourse.tile as tile
from concourse.bass_utils import run_bass_kernel_spmd

S = 1024          # sequence length
D = 4096          # model dim
H = 32            # query heads (global)
G = 8             # kv groups (global)
HD = 128          # head dim
N_CORES = 8
HPC = H // N_CORES   # 4 query heads per core
NDC = D // 128       # 32 contraction chunks
NCP = NDC // 2       # 16 chunk pairs
BF = mybir.dt.bfloat16
F32 = mybir.dt.float32
E4 = mybir.dt.float8e4
DR = mybir.MatmulPerfMode.DoubleRow

# host-side hi/lo scales (fp8 payloads are SCALE*true value)
SC_WQ = 512.0     # Wq with 1/sqrt(HD) folded  (sigma ~0.0018 -> ~0.9)
SC_WK = 64.0      # Wk/Wv/Wo sigma 0.02 -> ~1.28
SC_QH = 1.0       # qhat stored at true scale in bf16
_CACHE = {}


def _t_tiles(j):
    """Causal t-tile list for the 512-wide s-tile j, with mask index or None."""
    out = []
    for i in range(4 * j + 4):
        lo = i - 4 * j
        out.append((i, lo if 0 <= lo <= 3 else None))
    return out


def _build():
    nc = bacc.Bacc("TRN2", target_bir_lowering=False, debug=False,
                   num_devices=N_CORES)

    # layouts: hilo-major so every DoubleRow pair slice is contiguous
    xq = nc.dram_tensor("xq", [128, 2, 2, NDC, 512], E4, kind="ExternalInput")
    wq = nc.dram_tensor("wq", [128, 2, HPC, NDC, 128], E4, kind="ExternalInput")
    wk = nc.dram_tensor("wk", [128, 2, NDC, 128], E4, kind="ExternalInput")
    wv = nc.dram_tensor("wv", [128, 2, NDC, 128], E4, kind="ExternalInput")
    wo = nc.dram_tensor("wo", [128, 2, 8, HPC, 512], E4, kind="ExternalInput")
    cosT = nc.dram_tensor("cosT", [HD, S], BF, kind="ExternalInput")
    sinT = nc.dram_tensor("sinT", [HD, S], BF, kind="ExternalInput")
    rmat = nc.dram_tensor("rmat", [HD, HD], BF, kind="ExternalInput")
    masks = nc.dram_tensor("masks", [128, 128], BF, kind="ExternalInput")
    out = nc.dram_tensor("out", [S, D], BF, kind="ExternalOutput")

    with tile.TileContext(nc) as tc:
        _emit(tc, nc, xq, wq, wk, wv, wo, cosT, sinT, rmat, masks, out)
    nc.compile()
    return nc


def _emit(tc, nc, xq, wq, wk, wv, wo, cosT, sinT, rmat, masks, out):
    import contextlib
    ctx = contextlib.ExitStack()
    with ctx:
        const = ctx.enter_context(tc.tile_pool(name="const", bufs=1))
        work = ctx.enter_context(tc.tile_pool(name="work", bufs=1))
        tmp = ctx.enter_context(tc.tile_pool(name="tmp", bufs=4))
        pt_pool = ctx.enter_context(tc.tile_pool(name="pt", bufs=4))
        outp = ctx.enter_context(tc.tile_pool(name="outp", bufs=4))
        ps = ctx.enter_context(tc.tile_pool(name="ps", bufs=1, space="PSUM"))

        # ---- constants / weights into SBUF ----
        # DMA priority stream: wk, then x-hi(j0)‖wq-hi groups, wq-lo,
        # x-lo(j0), cos/sin/wv, x(j1), masks, wo.  The QKV j0 product runs
        # as three passes (hi*hi, lo*hi, hi*lo) so compute starts on x-hi
        # alone and each pass is fed just-in-time.
        rmat_sb = const.tile([HD, HD], BF, tag="rmat")
        ones_sb = const.tile([128, 1], BF, tag="ones")
        nc.vector.memset(ones_sb[:], 1.0)
        wk_sb = const.tile([128, 2, NDC, 128], E4, tag="wk")
        nc.sync.dma_start(out=wk_sb[:, :, 0:8, :], in_=wk.ap()[:, :, 0:8, :])
        x_sb = const.tile([128, 2, 2, NDC, 512], E4, tag="x")
        wq_sb = const.tile([128, 2, HPC, NDC, 128], E4, tag="wq")
        for c0, c1 in ((0, 4), (4, 8), (8, 16), (16, 24)):
            nc.sync.dma_start(out=x_sb[:, 0, 0, c0:c1, :],
                              in_=xq.ap()[:, 0, 0, c0:c1, :])
            nc.sync.dma_start(out=wq_sb[:, 0, :, c0:c1, :],
                              in_=wq.ap()[:, 0, :, c0:c1, :])
        nc.sync.dma_start(out=wq_sb[:, 1, :, 0:8, :],
                          in_=wq.ap()[:, 1, :, 0:8, :])
        nc.sync.dma_start(out=x_sb[:, 0, 0, 24:32, :],
                          in_=xq.ap()[:, 0, 0, 24:32, :])
        nc.sync.dma_start(out=wq_sb[:, 0, :, 24:32, :],
                          in_=wq.ap()[:, 0, :, 24:32, :])
        nc.sync.dma_start(out=wq_sb[:, 1, :, 8:16, :],
                          in_=wq.ap()[:, 1, :, 8:16, :])
        nc.sync.dma_start(out=wk_sb[:, :, 8:, :], in_=wk.ap()[:, :, 8:, :])
        nc.sync.dma_start(out=wq_sb[:, 1, :, 16:32, :],
                          in_=wq.ap()[:, 1, :, 16:32, :])
        nc.sync.dma_start(out=rmat_sb[:], in_=rmat.ap())
        for c0, c1 in ((0, 16), (16, 32)):
            nc.sync.dma_start(out=x_sb[:, 1, 0, c0:c1, :],
                              in_=xq.ap()[:, 1, 0, c0:c1, :])
        cos_sb = const.tile([HD, S], BF, tag="cos")
        nc.sync.dma_start(out=cos_sb[:], in_=cosT.ap())
        sin_sb = const.tile([HD, S], BF, tag="sin")
        nc.sync.dma_start(out=sin_sb[:], in_=sinT.ap())
        wv_sb = const.tile([128, 2, NDC, 128], E4, tag="wv")
        nc.sync.dma_start(out=wv_sb[:], in_=wv.ap())
        for hl in range(2):
            for c0, c1 in ((0, 16), (16, 32)):
                nc.sync.dma_start(out=x_sb[:, hl, 1, c0:c1, :],
                                  in_=xq.ap()[:, hl, 1, c0:c1, :])
        mask_sb = const.tile([128, 128], BF, tag="mask")
        nc.sync.dma_start(out=mask_sb[:], in_=masks.ap())
        wo_sb = const.tile([128, 2, 8, HPC, 512], E4, tag="wo")
        for hl in range(2):
            nc.sync.dma_start(out=wo_sb[:, hl], in_=wo.ap()[:, hl])

        # persistent activations
        khat = work.tile([HD, 8, 128], BF, tag="khat")       # per t-tile
        qhat = {(h, j): work.tile([HD, 512], BF, tag=f"qh{h}_{j}",
                                  name=f"qh{h}_{j}")
                for h in range(HPC) for j in range(2)}
        v_sb = work.tile([128, 8, HD], BF, tag="v")          # [t, tile, hd]
        # ctx hi/lo fp8: [hd, hilo, q(8), h, 128] - head pairs contiguous
        ctxq = work.tile([HD, 2, 8, HPC, 128], E4, tag="ctxq")

        def mmt(acc, w_ap, x_ap, first=False, last=False):
            nc.tensor.matmul(acc, w_ap, x_ap, start=first, stop=last,
                             perf_mode=DR)

        def rope_rest(dst, raw, j):
            rq = ps.tile([HD, 512], F32, tag="ps", name="ps", bufs=4)
            nc.tensor.matmul(rq[:], rmat_sb[:], raw[:], start=True, stop=True)
            t1 = tmp.tile([HD, 512], BF, tag="rope_t1", name="rope_t1", bufs=2)
            nc.vector.tensor_mul(t1[:], raw[:], cos_sb[:, j * 512:(j + 1) * 512])
            t2 = tmp.tile([HD, 512], BF, tag="rope_t2", name="rope_t2", bufs=2)
            nc.vector.tensor_mul(t2[:], rq[:], sin_sb[:, j * 512:(j + 1) * 512])
            nc.vector.tensor_add(dst, t1[:], t2[:])

        def v_chain(j, ti):
            i = 4 * j + ti
            sl = slice(ti * 128, (ti + 1) * 128)
            vp = ps.tile([128, HD], F32, tag="ps", name="vp", bufs=4)
            for cp in range(NCP):
                c = 2 * cp
                xh = x_sb[:, 0, j, c:c + 2, sl]
                xl = x_sb[:, 1, j, c:c + 2, sl]
                mmt(vp[:], xh, wv_sb[:, 0, c:c + 2, :], first=(cp == 0))
                mmt(vp[:], xl, wv_sb[:, 0, c:c + 2, :])
                mmt(vp[:], xh, wv_sb[:, 1, c:c + 2, :], last=(cp == NCP - 1))
            nc.vector.tensor_scalar_mul(v_sb[:, i, :], vp[:], 1.0 / SC_WK)

        # ---- QKV projections + RoPE ----
        def qkv_phase(j):
            kp = ps.tile([HD, 512], F32, tag="ps", name="kp", bufs=4)
            qp2 = [ps.tile([HD, 1024], F32, tag="ps2", name=f"qp2_{m}", bufs=2)
                   for m in range(2)]
            qps = [qp2[h // 2][:, (h % 2) * 512:(h % 2) * 512 + 512]
                   for h in range(HPC)]
            # pass 1: Whi x Xhi   pass 2: Wlo x Xhi   pass 3: Whi x Xlo
            for whl, xhl, first, last in ((0, 0, True, False),
                                          (1, 0, False, False),
                                          (0, 1, False, True)):
                for cp in range(NCP):
                    c = 2 * cp
                    f = first and cp == 0
                    l = last and cp == NCP - 1
                    xap = x_sb[:, xhl, j, c:c + 2, :]
                    mmt(kp[:], wk_sb[:, whl, c:c + 2, :], xap, f, l)
                    for h in range(HPC):
                        mmt(qps[h], wq_sb[:, whl, h, c:c + 2, :], xap, f, l)
            # rope chains interleaved with this half's V chains; raw copies
            # alternate DVE/ACT so the PE rope matmuls aren't gated on one
            # engine's queue
            raw = tmp.tile([HD, 512], BF, tag="rk", name="rk", bufs=2)
            nc.vector.tensor_scalar_mul(raw[:], kp[:], 1.0 / SC_WK)
            v_chain(j, 0)
            rope_rest(khat[:, 4 * j:4 * j + 4, :], raw, j)
            for h in range(HPC):
                rawq = tmp.tile([HD, 512], BF, tag="rq", name="rq", bufs=2)
                if h % 2 == 0:
                    nc.scalar.activation(rawq[:], qps[h],
                                         mybir.ActivationFunctionType.Copy,
                                         scale=SC_QH / SC_WQ)
                else:
                    nc.vector.tensor_scalar_mul(rawq[:], qps[h],
                                                SC_QH / SC_WQ)
                if h < 3:
                    v_chain(j, h + 1)
                rope_rest(qhat[(h, j)][:], rawq, j)

        # ---- out-proj helpers ----
        ot_half = {}

        def op_chain(st_i, n, eng):
            """512-wide out-proj chain + staged copy; DMA per 1024 block."""
            opp = ps.tile([128, 512], F32, tag="ps", name="opp", bufs=4)
            for mp in range(2):
                hs = slice(2 * mp, 2 * mp + 2)
                mmt(opp[:], ctxq[:, 0, st_i, hs, :], wo_sb[:, 0, n, hs, :],
                    first=(mp == 0))
                mmt(opp[:], ctxq[:, 1, st_i, hs, :], wo_sb[:, 0, n, hs, :])
                mmt(opp[:], ctxq[:, 0, st_i, hs, :], wo_sb[:, 1, n, hs, :],
                    last=(mp == 1))
            n2, sub = divmod(n, 2)
            if sub == 0:
                ot_half[(st_i, n2)] = outp.tile([128, 1024], BF, tag="ot",
                                                name="ot")
            ot = ot_half[(st_i, n2)]
            dst = ot[:, sub * 512:(sub + 1) * 512]
            if eng == 0:
                nc.vector.tensor_scalar_mul(dst, opp[:], 1.0 / SC_WK)
            else:
                nc.scalar.activation(dst, opp[:],
                                     mybir.ActivationFunctionType.Copy,
                                     scale=1.0 / SC_WK)
            if sub == 1:
                nc.sync.dma_start(
                    out=out.ap()[st_i * 128:(st_i + 1) * 128,
                                 n2 * 1024:(n2 + 1) * 1024],
                    in_=ot[:])

        # ---- attention, software-pipelined, with PE fill hooks ----
        def attn_phase(j, take):
            """Attention for s-half j with exact-causal 128-wide s-tiles.
            Each s-tile q attends t-tiles 0..q; scores pack into one 2-bank
            psum, one batched exp per s-tile; den/cx accumulate as four
            bank-chained [.,128] chains per head.  take(k) emits queued
            filler PE chains."""
            for h in range(HPC):
                pts = {}

                def scores_q(q):
                    gq = 4 * j + q
                    nt = gq + 1
                    st = ps.tile([128, 1024], F32, tag="ps2", name="st",
                                 bufs=2)
                    qs = qhat[(h, j)][:, q * 128:(q + 1) * 128]
                    for t in range(nt):
                        nc.tensor.matmul(st[:, t * 128:(t + 1) * 128],
                                         khat[:, t, :], qs,
                                         start=True, stop=True)
                    pt = pt_pool.tile([128, 1024], BF, tag="pt", name="pt")
                    nc.scalar.activation(pt[:, 0:nt * 128], st[:, 0:nt * 128],
                                         mybir.ActivationFunctionType.Exp)
                    dd = slice((nt - 1) * 128, nt * 128)
                    nc.gpsimd.tensor_mul(pt[:, dd], pt[:, dd], mask_sb[:])
                    pts[q] = (pt, nt)

                scores_q(0)
                scores_q(1)
                if j == 1:
                    take(2)
                den = ps.tile([1, 512], F32, tag="ps", name="den", bufs=4)
                cx = ps.tile([HD, 512], F32, tag="ps", name="cx", bufs=4)
                for q in range(4):
                    pt, nt = pts[q]
                    qq = slice(q * 128, (q + 1) * 128)
                    for t in range(nt):
                        nc.tensor.matmul(den[0:1, qq], ones_sb[:],
                                         pt[:, t * 128:(t + 1) * 128],
                                         start=(t == 0), stop=(t == nt - 1))
                    for t in range(nt):
                        nc.tensor.matmul(cx[:, qq], v_sb[:, t, :],
                                         pt[:, t * 128:(t + 1) * 128],
                                         start=(t == 0), stop=(t == nt - 1))
                    if q + 2 < 4:
                        scores_q(q + 2)
                    if j == 1:
                        take(2 if q >= 2 else 1)
                finish_head(j, h, den, cx)

        def finish_head(j, h, den, cx):
            rec = tmp.tile([1, 512], F32, tag="rec", name="rec", bufs=1)
            nc.vector.reciprocal(rec[:], den[:])
            bc = tmp.tile([128, 512], F32, tag="bc", name="bc", bufs=2)
            nc.gpsimd.partition_broadcast(bc[:], rec[:])
            ctxf = tmp.tile([HD, 512], F32, tag="cxf", name="cxf", bufs=2)
            nc.vector.tensor_mul(ctxf[:], cx[:], bc[:])
            dst_h = ctxq[:, 0, 4 * j:4 * j + 4, h, :]
            dst_l = ctxq[:, 1, 4 * j:4 * j + 4, h, :]
            nc.vector.tensor_copy(dst_h, ctxf[:])
            nc.vector.tensor_sub(dst_l, ctxf[:], dst_h)

        # ---- emission schedule ----
        qkv_phase(0)
        qkv_phase(1)
        attn_phase(0, lambda k: None)
        # attn j1: fill PE waits with the 32 out-proj(j0) chains
        def eng_of(n):
            return 1 if n % 4 == 3 else 0
        opq = [(s, n) for s in range(4) for n in range(8)]

        def take(k):
            for _ in range(min(k, len(opq))):
                s, n = opq.pop(0)
                op_chain(s, n, eng_of(n))
        attn_phase(1, take)
        take(len(opq))
        # out-proj j1 (dedicated phase, 1024-wide psum chains); the first two
        # tiles' head-pair-0 half chains are emitted up front - they depend
        # only on j1 heads 0/1 and bridge the wait for head 3's ctx
        pre_ops = {}
        for n2 in range(2):
            op = ps.tile([128, 1024], F32, tag="ps2", name="op", bufs=2)
            pre_ops[n2] = op
            for sub in range(2):
                n = 2 * n2 + sub
                dst = op[:, sub * 512:(sub + 1) * 512]
                hs = slice(0, 2)
                mmt(dst, ctxq[:, 0, 4, hs, :], wo_sb[:, 0, n, hs, :],
                    first=True)
                mmt(dst, ctxq[:, 1, 4, hs, :], wo_sb[:, 0, n, hs, :])
                mmt(dst, ctxq[:, 0, 4, hs, :], wo_sb[:, 1, n, hs, :])
        for q in range(4):
            st_i = 4 + q
            for n2 in range(4):
                op = pre_ops.pop(n2, None) if q == 0 else None
                mps = (1,) if op is not None else (0, 1)
                if op is None:
                    op = ps.tile([128, 1024], F32, tag="ps2", name="op",
                                 bufs=2)
                for sub in range(2):
                    n = 2 * n2 + sub
                    dst = op[:, sub * 512:(sub + 1) * 512]
                    for mp in mps:
                        hs = slice(2 * mp, 2 * mp + 2)
                        mmt(dst, ctxq[:, 0, st_i, hs, :],
                            wo_sb[:, 0, n, hs, :], first=(mp == 0))
                        mmt(dst, ctxq[:, 1, st_i, hs, :],
                            wo_sb[:, 0, n, hs, :])
                        mmt(dst, ctxq[:, 0, st_i, hs, :],
                            wo_sb[:, 1, n, hs, :], last=(mp == 1))
                ot = outp.tile([128, 1024], BF, tag="ot", name="ot")
                if q == 3 and n2 == 2:
                    nc.vector.tensor_scalar_mul(ot[:, 0:512], op[:, 0:512],
                                                1.0 / SC_WK)
                    nc.scalar.activation(ot[:, 512:1024], op[:, 512:1024],
                                         mybir.ActivationFunctionType.Copy,
                                         scale=1.0 / SC_WK)
                elif q == 3 and n2 == 3:
                    # final tile: copy+ship halves independently on both
                    # engines so the tail is one 512-block deep
                    nc.vector.tensor_scalar_mul(ot[:, 0:512], op[:, 0:512],
                                                1.0 / SC_WK)
                    nc.sync.dma_start(
                        out=out.ap()[st_i * 128:(st_i + 1) * 128,
                                     n2 * 1024:n2 * 1024 + 512],
                        in_=ot[:, 0:512])
                    nc.scalar.activation(ot[:, 512:1024], op[:, 512:1024],
                                         mybir.ActivationFunctionType.Copy,
                                         scale=1.0 / SC_WK)
                    nc.sync.dma_start(
                        out=out.ap()[st_i * 128:(st_i + 1) * 128,
                                     n2 * 1024 + 512:(n2 + 1) * 1024],
                        in_=ot[:, 512:1024])
                    continue
                elif n2 % 2 == 0:
                    nc.vector.tensor_scalar_mul(ot[:], op[:], 1.0 / SC_WK)
                else:
                    nc.scalar.activation(ot[:], op[:],
                                         mybir.ActivationFunctionType.Copy,
                                         scale=1.0 / SC_WK)
                nc.sync.dma_start(
                    out=out.ap()[st_i * 128:(st_i + 1) * 128,
                                 n2 * 1024:(n2 + 1) * 1024],
                    in_=ot[:])


def _hilo(a):
    """e4m3 hi+lo decomposition (fp32 in, two e4m3 out)."""
    e4np = ml_dtypes.float8_e4m3
    hi = np.clip(a, -240, 240).astype(e4np)
    lo = np.clip(a - hi.astype(np.float32), -240, 240).astype(e4np)
    return hi, lo


def _prep_inputs(x, cos, sin, Wq, Wk, Wv, Wo):
    """Host-side shard + layout prep. Returns per-core input maps."""
    bf = ml_dtypes.bfloat16
    x2 = np.asarray(x, np.float32).reshape(S, D)

    # x: [128, hilo, j, c, s]
    xT = np.ascontiguousarray(x2.T)                      # [D, S]
    xh, xl = _hilo(xT)
    xq = np.stack([xh, xl], axis=0).reshape(2, NDC, 128, 2, 512)
    xq = np.ascontiguousarray(xq.transpose(2, 0, 3, 1, 4))  # [128,2,2,NDC,512]

    cosT = np.ascontiguousarray(np.asarray(cos, np.float32).T).astype(bf)
    sinT = np.ascontiguousarray(np.asarray(sin, np.float32).T).astype(bf)

    rmat = np.zeros((HD, HD), np.float32)
    half = HD // 2
    rmat[np.arange(half), np.arange(half) + half] = 1.0
    rmat[np.arange(half) + half, np.arange(half)] = -1.0
    rmat = rmat.astype(bf)

    lt = np.arange(128)[:, None]
    ls = np.arange(128)[None, :]
    masks = np.ascontiguousarray(lt <= ls).astype(bf)

    scale = 1.0 / np.sqrt(np.float32(HD))
    Wq = np.asarray(Wq, np.float32) * (scale * SC_WQ)
    Wk = np.asarray(Wk, np.float32) * SC_WK
    Wv = np.asarray(Wv, np.float32) * SC_WK
    Wo = np.asarray(Wo, np.float32) * SC_WK

    in_maps = []
    for r in range(N_CORES):
        # wq: [128, hilo, h, c, col]
        wq_r = Wq[:, r * HPC * HD:(r + 1) * HPC * HD]     # [D, 512]
        h_, l_ = _hilo(wq_r)
        wq_p = np.stack([h_, l_], 0).reshape(2, NDC, 128, HPC, 128)
        wq_p = np.ascontiguousarray(wq_p.transpose(2, 0, 3, 1, 4))

        def kv_pack(w):                                   # [D,128]->[128,2,NDC,128]
            h2, l2 = _hilo(w)
            p = np.stack([h2, l2], 0).reshape(2, NDC, 128, 128)
            return np.ascontiguousarray(p.transpose(2, 0, 1, 3))

        wk_p = kv_pack(Wk[:, r * HD:(r + 1) * HD])
        wv_p = kv_pack(Wv[:, r * HD:(r + 1) * HD])

        # wo: [128, hilo, n, h, col]; rows r*512..(r+1)*512 of Wo
        wo_r = Wo[r * HPC * HD:(r + 1) * HPC * HD, :]     # [512, D]
        h_, l_ = _hilo(wo_r)
        wo_p = np.stack([h_, l_], 0).reshape(2, HPC, 128, 8, 512)
        wo_p = np.ascontiguousarray(wo_p.transpose(2, 0, 3, 1, 4))

        in_maps.append({
            "xq": xq, "wq": wq_p, "wk": wk_p, "wv": wv_p, "wo": wo_p,
            "cosT": cosT, "sinT": sinT, "rmat": rmat, "masks": masks,
        })
    return in_maps


def get_nc():
    if "nc" not in _CACHE:
        _CACHE["nc"] = _build()
    return _CACHE["nc"]


def kernel(x, mask, cos, sin, Wq, Wk, Wv, Wo):
    nc = get_nc()
    in_maps = _prep_inputs(x, cos, sin, Wq, Wk, Wv, Wo)
    res = run_bass_kernel_spmd(nc, in_maps, core_ids=list(range(N_CORES)))
    acc = np.zeros((S, D), np.float32)
    for r in range(N_CORES):
        acc += res.results[r]["out"].astype(np.float32)
    return acc[None]


if __name__ == "__main__":
    print("built:", get_nc() is not None)


# revision 21
# speedup vs baseline: 1.2196x; 1.0132x over previous
"""Grouped-query attention, tensor-parallel over heads across 8 TRN2 NeuronCores.

Problem (hardcoded): x[1,1024,4096] @ Wq/Wk/Wv -> RoPE -> causal GQA
(32 q heads, 8 kv groups, head_dim 128) -> out proj Wo -> [1,1024,4096].

Sharding: core r owns q heads 4r..4r+3 and kv group r (Wq/Wk/Wv column
shards, Wo row shard). Each core computes a full [1024,4096] partial of
the output projection; the host sums the 8 partials (the "all-reduce").

Projections run as fp8 hi-lo DoubleRow matmuls: every operand is split
into e4m3 hi + lo parts (combined quantization error ~0.1%, better than
bf16) and each product is computed with three DoubleRow matmuls per pair
of 128-deep contraction chunks (hi*hi, lo*hi, hi*lo; the dropped lo*lo
term is ~0.06%).  DoubleRow contracts 256 rows per pass at half the
per-column cost of bf16, so projections run at 0.75x the bf16 cycle
count.  Attention (scores/softmax/ctx) stays bf16.
"""

import numpy as np
import ml_dtypes

import concourse.bass as bass
import concourse.bacc as bacc
import concourse.mybir as mybir
import concourse.tile as tile
from concourse.bass_utils import run_bass_kernel_spmd

S = 1024          # sequence length
D = 4096          # model dim
H = 32            # query heads (global)
G = 8             # kv groups (global)
HD = 128          # head dim
N_CORES = 8
HPC = H // N_CORES   # 4 query heads per core
NDC = D // 128       # 32 contraction chunks
NCP = NDC // 2       # 16 chunk pairs
BF = mybir.dt.bfloat16
F32 = mybir.dt.float32
E4 = mybir.dt.float8e4
DR = mybir.MatmulPerfMode.DoubleRow

# host-side hi/lo scales (fp8 payloads are SCALE*true value)
SC_WQ = 512.0     # Wq with 1/sqrt(HD) folded  (sigma ~0.0018 -> ~0.9)
SC_WK = 64.0      # Wk/Wv/Wo sigma 0.02 -> ~1.28
SC_QH = 1.0       # qhat stored at true scale in bf16
_CACHE = {}


def _t_tiles(j):
    """Causal t-tile list for the 512-wide s-tile j, with mask index or None."""
    out = []
    for i in range(4 * j + 4):
        lo = i - 4 * j
        out.append((i, lo if 0 <= lo <= 3 else None))
    return out


def _build():
    nc = bacc.Bacc("TRN2", target_bir_lowering=False, debug=False,
                   num_devices=N_CORES)

    # layouts: hilo-major so every DoubleRow pair slice is contiguous
    xq = nc.dram_tensor("xq", [128, 2, 2, NDC, 512], E4, kind="ExternalInput")
    wq = nc.dram_tensor("wq", [128, 2, HPC, NDC, 128], E4, kind="ExternalInput")
    wk = nc.dram_tensor("wk", [128, 2, NDC, 128], E4, kind="ExternalInput")
    wv = nc.dram_tensor("wv", [128, 2, NDC, 128], E4, kind="ExternalInput")
    wo = nc.dram_tensor("wo", [128, 2, 8, HPC, 512], E4, kind="ExternalInput")
    cosT = nc.dram_tensor("cosT", [HD, S], BF, kind="ExternalInput")
    sinT = nc.dram_tensor("sinT", [HD, S], BF, kind="ExternalInput")
    rmat = nc.dram_tensor("rmat", [HD, HD], BF, kind="ExternalInput")
    masks = nc.dram_tensor("masks", [128, 128], BF, kind="ExternalInput")
    out = nc.dram_tensor("out", [S, D], BF, kind="ExternalOutput")

    with tile.TileContext(nc) as tc:
        _emit(tc, nc, xq, wq, wk, wv, wo, cosT, sinT, rmat, masks, out)
    nc.compile()
    return nc


def _emit(tc, nc, xq, wq, wk, wv, wo, cosT, sinT, rmat, masks, out):
    import contextlib
    ctx = contextlib.ExitStack()
    with ctx:
        const = ctx.enter_context(tc.tile_pool(name="const", bufs=1))
        work = ctx.enter_context(tc.tile_pool(name="work", bufs=1))
        tmp = ctx.enter_context(tc.tile_pool(name="tmp", bufs=4))
        pt_pool = ctx.enter_context(tc.tile_pool(name="pt", bufs=4))
        outp = ctx.enter_context(tc.tile_pool(name="outp", bufs=4))
        ps = ctx.enter_context(tc.tile_pool(name="ps", bufs=1, space="PSUM"))

        # ---- constants / weights into SBUF ----
        # DMA priority stream: wk, then x-hi(j0)‖wq-hi groups, wq-lo,
        # x-lo(j0), cos/sin/wv, x(j1), masks, wo.  The QKV j0 product runs
        # as three passes (hi*hi, lo*hi, hi*lo) so compute starts on x-hi
        # alone and each pass is fed just-in-time.
        rmat_sb = const.tile([HD, HD], BF, tag="rmat")
        ones_sb = const.tile([128, 1], BF, tag="ones")
        nc.vector.memset(ones_sb[:], 1.0)
        wk_sb = const.tile([128, 2, NDC, 128], E4, tag="wk")
        nc.sync.dma_start(out=wk_sb[:, :, 0:8, :], in_=wk.ap()[:, :, 0:8, :])
        x_sb = const.tile([128, 2, 2, NDC, 512], E4, tag="x")
        wq_sb = const.tile([128, 2, HPC, NDC, 128], E4, tag="wq")
        for c0, c1 in ((0, 4), (4, 8), (8, 16), (16, 24)):
            nc.sync.dma_start(out=x_sb[:, 0, 0, c0:c1, :],
                              in_=xq.ap()[:, 0, 0, c0:c1, :])
            nc.sync.dma_start(out=wq_sb[:, 0, :, c0:c1, :],
                              in_=wq.ap()[:, 0, :, c0:c1, :])
        nc.sync.dma_start(out=wq_sb[:, 1, :, 0:8, :],
                          in_=wq.ap()[:, 1, :, 0:8, :])
        nc.sync.dma_start(out=x_sb[:, 0, 0, 24:32, :],
                          in_=xq.ap()[:, 0, 0, 24:32, :])
        nc.sync.dma_start(out=wq_sb[:, 0, :, 24:32, :],
                          in_=wq.ap()[:, 0, :, 24:32, :])
        nc.sync.dma_start(out=wq_sb[:, 1, :, 8:16, :],
                          in_=wq.ap()[:, 1, :, 8:16, :])
        nc.sync.dma_start(out=wk_sb[:, :, 8:, :], in_=wk.ap()[:, :, 8:, :])
        nc.sync.dma_start(out=wq_sb[:, 1, :, 16:32, :],
                          in_=wq.ap()[:, 1, :, 16:32, :])
        nc.sync.dma_start(out=rmat_sb[:], in_=rmat.ap())
        for c0, c1 in ((0, 16), (16, 32)):
            nc.sync.dma_start(out=x_sb[:, 1, 0, c0:c1, :],
                              in_=xq.ap()[:, 1, 0, c0:c1, :])
        cos_sb = const.tile([HD, S], BF, tag="cos")
        nc.sync.dma_start(out=cos_sb[:], in_=cosT.ap())
        sin_sb = const.tile([HD, S], BF, tag="sin")
        nc.sync.dma_start(out=sin_sb[:], in_=sinT.ap())
        wv_sb = const.tile([128, 2, NDC, 128], E4, tag="wv")
        nc.sync.dma_start(out=wv_sb[:], in_=wv.ap())
        for hl in range(2):
            for c0, c1 in ((0, 16), (16, 32)):
                nc.sync.dma_start(out=x_sb[:, hl, 1, c0:c1, :],
                                  in_=xq.ap()[:, hl, 1, c0:c1, :])
        mask_sb = const.tile([128, 128], BF, tag="mask")
        nc.sync.dma_start(out=mask_sb[:], in_=masks.ap())
        wo_sb = const.tile([128, 2, 8, HPC, 512], E4, tag="wo")
        for hl in range(2):
            nc.sync.dma_start(out=wo_sb[:, hl], in_=wo.ap()[:, hl])

        # persistent activations
        khat = work.tile([HD, 8, 128], BF, tag="khat")       # per t-tile
        qhat = {(h, j): work.tile([HD, 512], BF, tag=f"qh{h}_{j}",
                                  name=f"qh{h}_{j}")
                for h in range(HPC) for j in range(2)}
        v_sb = work.tile([128, 8, HD], BF, tag="v")          # [t, tile, hd]
        # ctx hi/lo fp8: [hd, hilo, q(8), h, 128] - head pairs contiguous
        ctxq = work.tile([HD, 2, 8, HPC, 128], E4, tag="ctxq")

        def mmt(acc, w_ap, x_ap, first=False, last=False):
            nc.tensor.matmul(acc, w_ap, x_ap, start=first, stop=last,
                             perf_mode=DR)

        def rope_rest(dst, raw, j):
            rq = ps.tile([HD, 512], F32, tag="ps", name="ps", bufs=4)
            nc.tensor.matmul(rq[:], rmat_sb[:], raw[:], start=True, stop=True)
            t1 = tmp.tile([HD, 512], BF, tag="rope_t1", name="rope_t1", bufs=2)
            nc.vector.tensor_mul(t1[:], raw[:], cos_sb[:, j * 512:(j + 1) * 512])
            t2 = tmp.tile([HD, 512], BF, tag="rope_t2", name="rope_t2", bufs=2)
            nc.vector.tensor_mul(t2[:], rq[:], sin_sb[:, j * 512:(j + 1) * 512])
            nc.vector.tensor_add(dst, t1[:], t2[:])

        def v_chain(j, ti):
            i = 4 * j + ti
            sl = slice(ti * 128, (ti + 1) * 128)
            vp = ps.tile([128, HD], F32, tag="ps", name="vp", bufs=4)
            for cp in range(NCP):
                c = 2 * cp
                xh = x_sb[:, 0, j, c:c + 2, sl]
                xl = x_sb[:, 1, j, c:c + 2, sl]
                mmt(vp[:], xh, wv_sb[:, 0, c:c + 2, :], first=(cp == 0))
                mmt(vp[:], xl, wv_sb[:, 0, c:c + 2, :])
                mmt(vp[:], xh, wv_sb[:, 1, c:c + 2, :], last=(cp == NCP - 1))
            nc.vector.tensor_scalar_mul(v_sb[:, i, :], vp[:], 1.0 / SC_WK)

        # ---- QKV projections + RoPE ----
        def qkv_phase(j):
            kp = ps.tile([HD, 512], F32, tag="ps", name="kp", bufs=4)
            qp2 = [ps.tile([HD, 1024], F32, tag="ps2", name=f"qp2_{m}", bufs=2)
                   for m in range(2)]
            qps = [qp2[h // 2][:, (h % 2) * 512:(h % 2) * 512 + 512]
                   for h in range(HPC)]
            # pass 1: Whi x Xhi   pass 2: Wlo x Xhi   pass 3: Whi x Xlo
            for whl, xhl, first, last in ((0, 0, True, False),
                                          (1, 0, False, False),
                                          (0, 1, False, True)):
                for cp in range(NCP):
                    c = 2 * cp
                    f = first and cp == 0
                    l = last and cp == NCP - 1
                    xap = x_sb[:, xhl, j, c:c + 2, :]
                    mmt(kp[:], wk_sb[:, whl, c:c + 2, :], xap, f, l)
                    for h in range(HPC):
                        mmt(qps[h], wq_sb[:, whl, h, c:c + 2, :], xap, f, l)
            # rope chains interleaved with this half's V chains; raw copies
            # alternate DVE/ACT so the PE rope matmuls aren't gated on one
            # engine's queue
            raw = tmp.tile([HD, 512], BF, tag="rk", name="rk", bufs=2)
            nc.vector.tensor_scalar_mul(raw[:], kp[:], 1.0 / SC_WK)
            v_chain(j, 0)
            rope_rest(khat[:, 4 * j:4 * j + 4, :], raw, j)
            for h in range(HPC):
                rawq = tmp.tile([HD, 512], BF, tag="rq", name="rq", bufs=2)
                if h % 2 == 0:
                    nc.scalar.activation(rawq[:], qps[h],
                                         mybir.ActivationFunctionType.Copy,
                                         scale=SC_QH / SC_WQ)
                else:
                    nc.vector.tensor_scalar_mul(rawq[:], qps[h],
                                                SC_QH / SC_WQ)
                if h < 3:
                    v_chain(j, h + 1)
                rope_rest(qhat[(h, j)][:], rawq, j)

        # ---- out-proj helpers ----
        ot_half = {}

        def op_chain(st_i, n, eng):
            """512-wide out-proj chain + staged copy; DMA per 1024 block."""
            opp = ps.tile([128, 512], F32, tag="ps", name="opp", bufs=4)
            for mp in range(2):
                hs = slice(2 * mp, 2 * mp + 2)
                mmt(opp[:], ctxq[:, 0, st_i, hs, :], wo_sb[:, 0, n, hs, :],
                    first=(mp == 0))
                mmt(opp[:], ctxq[:, 1, st_i, hs, :], wo_sb[:, 0, n, hs, :])
                mmt(opp[:], ctxq[:, 0, st_i, hs, :], wo_sb[:, 1, n, hs, :],
                    last=(mp == 1))
            n2, sub = divmod(n, 2)
            if sub == 0:
                ot_half[(st_i, n2)] = outp.tile([128, 1024], BF, tag="ot",
                                                name="ot")
            ot = ot_half[(st_i, n2)]
            dst = ot[:, sub * 512:(sub + 1) * 512]
            if eng == 0:
                nc.vector.tensor_scalar_mul(dst, opp[:], 1.0 / SC_WK)
            else:
                nc.scalar.activation(dst, opp[:],
                                     mybir.ActivationFunctionType.Copy,
                                     scale=1.0 / SC_WK)
            if sub == 1:
                nc.sync.dma_start(
                    out=out.ap()[st_i * 128:(st_i + 1) * 128,
                                 n2 * 1024:(n2 + 1) * 1024],
                    in_=ot[:])

        # ---- attention, software-pipelined, with PE fill hooks ----
        def attn_phase(j, take):
            """Attention for s-half j with exact-causal 128-wide s-tiles.
            Each s-tile q attends t-tiles 0..q; scores pack into one 2-bank
            psum, one batched exp per s-tile; den/cx accumulate as four
            bank-chained [.,128] chains per head.  take(k) emits queued
            filler PE chains."""
            carry = {}
            for h in range(HPC):
                pts = carry
                carry = {}

                def scores_q(q, h2=None, into=None):
                    dst = pts if into is None else into
                    h2 = h if h2 is None else h2
                    gq = 4 * j + q
                    nt = gq + 1
                    st = ps.tile([128, 1024], F32, tag="ps2", name="st",
                                 bufs=2)
                    qs = qhat[(h2, j)][:, q * 128:(q + 1) * 128]
                    for t in range(nt):
                        nc.tensor.matmul(st[:, t * 128:(t + 1) * 128],
                                         khat[:, t, :], qs,
                                         start=True, stop=True)
                    pt = pt_pool.tile([128, 1024], BF, tag="pt", name="pt")
                    nc.scalar.activation(pt[:, 0:nt * 128], st[:, 0:nt * 128],
                                         mybir.ActivationFunctionType.Exp)
                    dd = slice((nt - 1) * 128, nt * 128)
                    nc.gpsimd.tensor_mul(pt[:, dd], pt[:, dd], mask_sb[:])
                    dst[q] = (pt, 0, nt)

                def scores_pair(qa, h2=None, into=None):
                    """Two packed j0 s-tiles (qa, qa+1) in one psum + one exp."""
                    dst = pts if into is None else into
                    h2 = h if h2 is None else h2
                    na, nb = qa + 1, qa + 2
                    st = ps.tile([128, 1024], F32, tag="ps2", name="st",
                                 bufs=2)
                    pt = pt_pool.tile([128, 1024], BF, tag="pt", name="pt")
                    for q, base in ((qa, 0), (qa + 1, na)):
                        nt = q + 1
                        qs = qhat[(h2, 0)][:, q * 128:(q + 1) * 128]
                        for t in range(nt):
                            o = (base + t) * 128
                            nc.tensor.matmul(st[:, o:o + 128], khat[:, t, :],
                                             qs, start=True, stop=True)
                    tot = na + nb
                    nc.scalar.activation(pt[:, 0:tot * 128], st[:, 0:tot * 128],
                                         mybir.ActivationFunctionType.Exp)
                    for q, base in ((qa, 0), (qa + 1, na)):
                        dd = slice((base + q) * 128, (base + q + 1) * 128)
                        nc.gpsimd.tensor_mul(pt[:, dd], pt[:, dd], mask_sb[:])
                        dst[q] = (pt, base, q + 1)

                if j == 0:
                    if 0 not in pts:
                        scores_pair(0)
                    scores_pair(2)
                else:
                    if 0 not in pts:
                        scores_q(0)
                    if 1 not in pts:
                        scores_q(1)
                    take(2)
                den = ps.tile([1, 512], F32, tag="ps", name="den", bufs=4)
                cx = ps.tile([HD, 512], F32, tag="ps", name="cx", bufs=4)
                for q in range(4):
                    pt, base, nt = pts[q]
                    qq = slice(q * 128, (q + 1) * 128)
                    for t in range(nt):
                        o = (base + t) * 128
                        nc.tensor.matmul(den[0:1, qq], ones_sb[:],
                                         pt[:, o:o + 128],
                                         start=(t == 0), stop=(t == nt - 1))
                    for t in range(nt):
                        o = (base + t) * 128
                        nc.tensor.matmul(cx[:, qq], v_sb[:, t, :],
                                         pt[:, o:o + 128],
                                         start=(t == 0), stop=(t == nt - 1))
                    if j == 1 and q + 2 < 4:
                        scores_q(q + 2)
                    elif j == 0 and q == 3 and h < HPC - 1:
                        scores_pair(0, h2=h + 1, into=carry)
                    if j == 1:
                        take(2 if q >= 2 else 1)
                finish_head(j, h, den, cx)

        def finish_head(j, h, den, cx):
            rec = tmp.tile([1, 512], F32, tag="rec", name="rec", bufs=1)
            nc.vector.reciprocal(rec[:], den[:])
            bc = tmp.tile([128, 512], F32, tag="bc", name="bc", bufs=2)
            nc.gpsimd.partition_broadcast(bc[:], rec[:])
            ctxf = tmp.tile([HD, 512], F32, tag="cxf", name="cxf", bufs=2)
            nc.vector.tensor_mul(ctxf[:], cx[:], bc[:])
            dst_h = ctxq[:, 0, 4 * j:4 * j + 4, h, :]
            dst_l = ctxq[:, 1, 4 * j:4 * j + 4, h, :]
            nc.vector.tensor_copy(dst_h, ctxf[:])
            nc.vector.tensor_sub(dst_l, ctxf[:], dst_h)

        # ---- emission schedule ----
        qkv_phase(0)
        qkv_phase(1)
        attn_phase(0, lambda k: None)
        # attn j1: fill PE waits with the 32 out-proj(j0) chains
        def eng_of(n):
            return 1 if n % 4 == 3 else 0
        opq = [(s, n) for s in range(4) for n in range(8)]

        def take(k):
            for _ in range(min(k, len(opq))):
                s, n = opq.pop(0)
                op_chain(s, n, eng_of(n))
        attn_phase(1, take)
        take(len(opq))
        # out-proj j1 (dedicated phase, 1024-wide psum chains); the first two
        # tiles' head-pair-0 half chains are emitted up front - they depend
        # only on j1 heads 0/1 and bridge the wait for head 3's ctx
        pre_ops = {}
        for n2 in range(2):
            op = ps.tile([128, 1024], F32, tag="ps2", name="op", bufs=2)
            pre_ops[n2] = op
            for sub in range(2):
                n = 2 * n2 + sub
                dst = op[:, sub * 512:(sub + 1) * 512]
                hs = slice(0, 2)
                mmt(dst, ctxq[:, 0, 4, hs, :], wo_sb[:, 0, n, hs, :],
                    first=True)
                mmt(dst, ctxq[:, 1, 4, hs, :], wo_sb[:, 0, n, hs, :])
                mmt(dst, ctxq[:, 0, 4, hs, :], wo_sb[:, 1, n, hs, :])
        for q in range(4):
            st_i = 4 + q
            for n2 in range(4):
                op = pre_ops.pop(n2, None) if q == 0 else None
                mps = (1,) if op is not None else (0, 1)
                if op is None:
                    op = ps.tile([128, 1024], F32, tag="ps2", name="op",
                                 bufs=2)
                for sub in range(2):
                    n = 2 * n2 + sub
                    dst = op[:, sub * 512:(sub + 1) * 512]
                    for mp in mps:
                        hs = slice(2 * mp, 2 * mp + 2)
                        mmt(dst, ctxq[:, 0, st_i, hs, :],
                            wo_sb[:, 0, n, hs, :], first=(mp == 0))
                        mmt(dst, ctxq[:, 1, st_i, hs, :],
                            wo_sb[:, 0, n, hs, :])
                        mmt(dst, ctxq[:, 0, st_i, hs, :],
                            wo_sb[:, 1, n, hs, :], last=(mp == 1))
                ot = outp.tile([128, 1024], BF, tag="ot", name="ot")
                if q == 3 and n2 == 2:
                    nc.vector.tensor_scalar_mul(ot[:, 0:512], op[:, 0:512],
                                                1.0 / SC_WK)
                    nc.scalar.activation(ot[:, 512:1024], op[:, 512:1024],
                                         mybir.ActivationFunctionType.Copy,
                                         scale=1.0 / SC_WK)
                elif q == 3 and n2 == 3:
                    # final tile: copy+ship halves independently on both
                    # engines so the tail is one 512-block deep
                    nc.vector.tensor_scalar_mul(ot[:, 0:512], op[:, 0:512],
                                                1.0 / SC_WK)
                    nc.sync.dma_start(
                        out=out.ap()[st_i * 128:(st_i + 1) * 128,
                                     n2 * 1024:n2 * 1024 + 512],
                        in_=ot[:, 0:512])
                    nc.scalar.activation(ot[:, 512:1024], op[:, 512:1024],
                                         mybir.ActivationFunctionType.Copy,
                                         scale=1.0 / SC_WK)
                    nc.sync.dma_start(
                        out=out.ap()[st_i * 128:(st_i + 1) * 128,
                                     n2 * 1024 + 512:(n2 + 1) * 1024],
                        in_=ot[:, 512:1024])
                    continue
                elif n2 % 2 == 0:
                    nc.vector.tensor_scalar_mul(ot[:], op[:], 1.0 / SC_WK)
                else:
                    nc.scalar.activation(ot[:], op[:],
                                         mybir.ActivationFunctionType.Copy,
                                         scale=1.0 / SC_WK)
                nc.sync.dma_start(
                    out=out.ap()[st_i * 128:(st_i + 1) * 128,
                                 n2 * 1024:(n2 + 1) * 1024],
                    in_=ot[:])


def _hilo(a):
    """e4m3 hi+lo decomposition (fp32 in, two e4m3 out)."""
    e4np = ml_dtypes.float8_e4m3
    hi = np.clip(a, -240, 240).astype(e4np)
    lo = np.clip(a - hi.astype(np.float32), -240, 240).astype(e4np)
    return hi, lo


def _prep_inputs(x, cos, sin, Wq, Wk, Wv, Wo):
    """Host-side shard + layout prep. Returns per-core input maps."""
    bf = ml_dtypes.bfloat16
    x2 = np.asarray(x, np.float32).reshape(S, D)

    # x: [128, hilo, j, c, s]
    xT = np.ascontiguousarray(x2.T)                      # [D, S]
    xh, xl = _hilo(xT)
    xq = np.stack([xh, xl], axis=0).reshape(2, NDC, 128, 2, 512)
    xq = np.ascontiguousarray(xq.transpose(2, 0, 3, 1, 4))  # [128,2,2,NDC,512]

    cosT = np.ascontiguousarray(np.asarray(cos, np.float32).T).astype(bf)
    sinT = np.ascontiguousarray(np.asarray(sin, np.float32).T).astype(bf)

    rmat = np.zeros((HD, HD), np.float32)
    half = HD // 2
    rmat[np.arange(half), np.arange(half) + half] = 1.0
    rmat[np.arange(half) + half, np.arange(half)] = -1.0
    rmat = rmat.astype(bf)

    lt = np.arange(128)[:, None]
    ls = np.arange(128)[None, :]
    masks = np.ascontiguousarray(lt <= ls).astype(bf)

    scale = 1.0 / np.sqrt(np.float32(HD))
    Wq = np.asarray(Wq, np.float32) * (scale * SC_WQ)
    Wk = np.asarray(Wk, np.float32) * SC_WK
    Wv = np.asarray(Wv, np.float32) * SC_WK
    Wo = np.asarray(Wo, np.float32) * SC_WK

    in_maps = []
    for r in range(N_CORES):
        # wq: [128, hilo, h, c, col]
        wq_r = Wq[:, r * HPC * HD:(r + 1) * HPC * HD]     # [D, 512]
        h_, l_ = _hilo(wq_r)
        wq_p = np.stack([h_, l_], 0).reshape(2, NDC, 128, HPC, 128)
        wq_p = np.ascontiguousarray(wq_p.transpose(2, 0, 3, 1, 4))

        def kv_pack(w):                                   # [D,128]->[128,2,NDC,128]
            h2, l2 = _hilo(w)
            p = np.stack([h2, l2], 0).reshape(2, NDC, 128, 128)
            return np.ascontiguousarray(p.transpose(2, 0, 1, 3))

        wk_p = kv_pack(Wk[:, r * HD:(r + 1) * HD])
        wv_p = kv_pack(Wv[:, r * HD:(r + 1) * HD])

        # wo: [128, hilo, n, h, col]; rows r*512..(r+1)*512 of Wo
        wo_r = Wo[r * HPC * HD:(r + 1) * HPC * HD, :]     # [512, D]
        h_, l_ = _hilo(wo_r)
        wo_p = np.stack([h_, l_], 0).reshape(2, HPC, 128, 8, 512)
        wo_p = np.ascontiguousarray(wo_p.transpose(2, 0, 3, 1, 4))

        in_maps.append({
            "xq": xq, "wq": wq_p, "wk": wk_p, "wv": wv_p, "wo": wo_p,
            "cosT": cosT, "sinT": sinT, "rmat": rmat, "masks": masks,
        })
    return in_maps


def get_nc():
    if "nc" not in _CACHE:
        _CACHE["nc"] = _build()
    return _CACHE["nc"]


def kernel(x, mask, cos, sin, Wq, Wk, Wv, Wo):
    nc = get_nc()
    in_maps = _prep_inputs(x, cos, sin, Wq, Wk, Wv, Wo)
    res = run_bass_kernel_spmd(nc, in_maps, core_ids=list(range(N_CORES)))
    acc = np.zeros((S, D), np.float32)
    for r in range(N_CORES):
        acc += res.results[r]["out"].astype(np.float32)
    return acc[None]


if __name__ == "__main__":
    print("built:", get_nc() is not None)
